# revision 1
# baseline (speedup 1.0000x reference)
"""GCN (2-layer GCNConv + global max pool + MLP + log_softmax) on 8 trn2 cores.

Strategy (sharding_hint: partition nodes + incident edges, replicate weights):
  - Nodes are partitioned 6250/core (+22 pad nodes/core -> 6272 = 49 tiles of
    128). Within each core, nodes are sorted by degree (desc) so that the
    per-tile padded gather width J_t ~= the true degree.
  - Edges are grouped by dst; each core owns edges into its nodes. For each
    128-node tile the messages are fetched with dma_gather (int16 signed
    indices relative to a mid-table base row cover all 50176/50000 rows),
    giving [128 nodes, J, C] tiles which are reduced on DVE.
  - GCN normalization: agg = D^-1/2 (A+I) D^-1/2 h. Layer-1 folds
    dinv[src] into the DVE accumulate; the produced h1 is pre-scaled by
    dinv (h1' = dinv * relu(...)), so layer-2 accumulation is plain adds.
  - h1' shards are AllGathered into a replicated table; layer-2 gathers
    from it. Max pooling is a dma_gather per graph-partition from the local
    h2 shard + DVE max-reduce, scattered into a [513,256] table (indirect
    scatter handles per-core graph offsets), AllReduce(max), then the small
    MLP + log_softmax run replicated on every core.
"""

import numpy as np

import concourse.bass as bass
import concourse.bacc as bacc
import concourse.tile as tile
import concourse.mybir as mybir
from concourse import bass_utils
from concourse.masks import make_identity
from concourse._compat import cdiv

F32 = mybir.dt.float32
I16 = mybir.dt.int16
I32 = mybir.dt.int32

NEG_BIG = -1.0e38


# ---------------------------------------------------------------- host prep

def _wrap_idx(flat):
    """j-major flat int16 idx list [n] -> wrapped SBUF layout [128, n//16].

    dma_gather consumes idx i from wrapped[i % 16, i // 16]; the 16-row
    pattern is replicated to all 128 partitions.
    """
    n = len(flat)
    assert n % 128 == 0
    w = np.zeros((16, n // 16), np.int16)
    w[np.arange(n) % 16, np.arange(n) // 16] = flat
    return np.tile(w, (8, 1))


def prep(x, edge_index, batch, n_graphs, n_cores=8, j_cap=6, pool_cap=6,
         mid_base=True):
    """All index-space preprocessing. Returns (meta, per-core arrays)."""
    N = x.shape[0]
    NR = N // n_cores                      # real nodes per core
    LV = int(cdiv(NR, 128)) * 128          # padded nodes per core
    T = LV // 128                          # tiles per core
    NP = LV * n_cores                      # padded total
    BASE1 = N // 2 if mid_base else 0      # x-table base row
    BASE2 = NP // 2 if mid_base else 0     # h1-table base row
    assert max(N - BASE1, BASE1, NP - BASE2, BASE2, NR + 1) <= 32767

    src = np.concatenate([edge_index[0], np.arange(N, dtype=np.int64)])
    dst = np.concatenate([edge_index[1], np.arange(N, dtype=np.int64)])
    deg = np.bincount(dst, minlength=N).astype(np.int64)
    dinv = np.zeros(N, np.float32)
    nz = deg > 0
    dinv[nz] = 1.0 / np.sqrt(np.maximum(deg[nz], 1).astype(np.float32))

    # group edges by dst
    order = np.argsort(dst, kind="stable")
    src_s = src[order]
    starts = np.searchsorted(dst[order], np.arange(N))
    ends = np.searchsorted(dst[order], np.arange(N) + 1)

    # per-core degree-sorted permutation; perm[c][l] = orig id, -1 = pad
    perm = np.full((n_cores, LV), -1, np.int64)
    for c in range(n_cores):
        lo = NR * c
        perm[c, :NR] = np.argsort(-deg[lo:lo + NR], kind="stable") + lo
    perm_row = np.full(N, -1, np.int64)   # orig id -> permuted global row
    for c in range(n_cores):
        perm_row[perm[c, :NR]] = LV * c + np.arange(NR)

    # J_t per tile (max over cores), split into gather chunks of width <=j_cap
    Jt = np.zeros(T, np.int64)
    for c in range(n_cores):
        for t in range(T):
            ids = perm[c, t * 128:(t + 1) * 128]
            ids = ids[ids >= 0]
            if len(ids):
                Jt[t] = max(Jt[t], deg[ids].max())
    Jt = np.maximum(Jt, 1)
    chunks = [[j_cap] * (int(j) // j_cap) + ([int(j) % j_cap] if j % j_cap else [])
              for j in Jt]
    sumJ = int(Jt.sum())

    # pad rows: any pad node's permuted row (h1' there is forced to 0);
    # use the globally-last pad row.
    pad_row2 = NP - 1 if NP > N else None
    assert pad_row2 is not None, "need at least one pad node for L2 padding"

    # per-core slot tables
    per_core = []
    for c in range(n_cores):
        idx1 = np.zeros((sumJ * 128,), np.int16)
        idx2 = np.zeros((sumJ * 128,), np.int16)
        dinv1 = np.zeros((128, sumJ), np.float32)
        dinv_dst = np.zeros((128, T), np.float32)
        off = 0
        for t in range(T):
            J = int(Jt[t])
            for p in range(128):
                n = perm[c, t * 128 + p]
                if n >= 0:
                    dinv_dst[p, t] = dinv[n]
                    ss = src_s[starts[n]:ends[n]]
                    nj = len(ss)
                    sl = (off + np.arange(nj)) * 128 + p
                    idx1[sl] = (ss - BASE1).astype(np.int16)
                    idx2[sl] = (perm_row[ss] - BASE2).astype(np.int16)
                    dinv1[p, off:off + nj] = dinv[ss]
                else:
                    nj = 0
                # pad slots
                if nj < J:
                    sl = (off + np.arange(nj, J)) * 128 + p
                    idx1[sl] = 0            # dinv1 = 0 neutralizes
                    idx2[sl] = pad_row2 - BASE2   # zero row
            off += J
        assert off == sumJ

        # wrapped layout per gather chunk; each chunk gets one trailing
        # all-pad block: the gather's final descriptor flakily skips its
        # data write, so the last 128 slots are sacrificial and never read.
        pad_blk = np.zeros(128, np.int16)
        w1 = []
        w2 = []
        off = 0
        for t in range(T):
            for w in chunks[t]:
                blk = slice(off * 128, (off + w) * 128)
                w1.append(_wrap_idx(np.concatenate([idx1[blk], pad_blk])))
                w2.append(_wrap_idx(np.concatenate([idx2[blk], pad_blk])))
                off += w
        idx1_w = np.concatenate(w1, axis=1)
        idx2_w = np.concatenate(w2, axis=1)
        per_core.append(dict(idx1=idx1_w, idx2=idx2_w, dinv1=dinv1,
                             dinv_dst=dinv_dst))

    # pooling: per-core graph ranges + member lists (permuted-local rows)
    glo = np.zeros(n_cores, np.int64)
    Gc = np.zeros(n_cores, np.int64)
    for c in range(n_cores):
        b = batch[NR * c:NR * (c + 1)]
        glo[c] = b.min()
        Gc[c] = b.max() - b.min() + 1
    G_max = int(Gc.max())
    assert G_max <= 128
    # member lists
    members = []   # [core][local graph] -> list of local permuted rows
    for c in range(n_cores):
        b = batch[NR * c:NR * (c + 1)]
        loc = [[] for _ in range(G_max)]
        inv_l = np.empty(NR, np.int64)
        inv_l[perm[c, :NR] - NR * c] = np.arange(NR)
        for i in range(NR):
            loc[int(b[i] - glo[c])].append(int(inv_l[i]))
        members.append(loc)
    Jp = max(max(len(m) for m in loc) for loc in members)
    pool_chunks = [pool_cap] * (Jp // pool_cap) + \
        ([Jp % pool_cap] if Jp % pool_cap else [])
    PAD_POOL = LV  # row LV of h2_local = NEG_BIG
    for c in range(n_cores):
        flat = np.full((len(pool_chunks) and sum(pool_chunks)) * 128, PAD_POOL,
                       np.int16)
        loc = members[c]
        for p in range(128):
            mem = loc[p] if p < G_max else []
            for j, r in enumerate(mem):
                flat[j * 128 + p] = r
        w = []
        off = 0
        pad_blk0 = np.zeros(128, np.int16)
        for wdt in pool_chunks:
            blk = slice(off * 128, (off + wdt) * 128)
            w.append(_wrap_idx(np.concatenate([flat[blk], pad_blk0])))
            off += wdt
        per_core[c]["idxp"] = np.concatenate(w, axis=1)
        scat = np.full(128, n_graphs, np.int64)
        scat[:int(Gc[c])] = glo[c] + np.arange(int(Gc[c]))
        per_core[c]["scat_g"] = scat.astype(np.int32)[:, None]

    meta = dict(N=N, NP=NP, LV=LV, T=T, NC=n_cores, BASE1=BASE1, BASE2=BASE2,
                chunks=chunks, sumJ=sumJ, pool_chunks=pool_chunks,
                n_graphs=n_graphs)
    return meta, per_core


# ---------------------------------------------------------------- bass build

def build(meta, CIN, HID, HMLP, NCL, stage=5):
    """Build the SPMD Bass program. All per-core variation flows via inputs."""
    m = meta
    T, NC = m["T"], m["NC"]
    N, NP, LV = m["N"], m["NP"], m["LV"]
    chunks, pool_chunks = m["chunks"], m["pool_chunks"]
    sumJ = m["sumJ"]
    NG = m["n_graphs"]
    NGT = cdiv(NG, 128)          # pooled tiles (4)
    n_chunk_cols = sum(sum(w + 1 for w in cl) for cl in chunks)
    n_pool_cols = sum(w + 1 for w in pool_chunks)

    nc = bacc.Bacc("TRN2", target_bir_lowering=False, debug=False,
                   num_devices=NC)
    dt = mybir.dt

    # ---- inputs
    x_t = nc.dram_tensor("x", [N, CIN], F32, kind="ExternalInput")
    idx1_t = nc.dram_tensor("idx1", [128, n_chunk_cols * 8], I16,
                            kind="ExternalInput")
    idx2_t = nc.dram_tensor("idx2", [128, n_chunk_cols * 8], I16,
                            kind="ExternalInput")
    dinv1_t = nc.dram_tensor("dinv1", [128, sumJ], F32, kind="ExternalInput")
    dinvd_t = nc.dram_tensor("dinv_dst", [128, T], F32, kind="ExternalInput")
    idxp_t = nc.dram_tensor("idxp", [128, n_pool_cols * 8], I16,
                            kind="ExternalInput")
    scat_t = nc.dram_tensor("scat_g", [128, 1], I32, kind="ExternalInput")
    W1_t = nc.dram_tensor("W1", [CIN, HID], F32, kind="ExternalInput")
    b1_t = nc.dram_tensor("b1", [1, HID], F32, kind="ExternalInput")
    W2_t = nc.dram_tensor("W2", [HID, HID], F32, kind="ExternalInput")
    b2_t = nc.dram_tensor("b2", [1, HID], F32, kind="ExternalInput")
    fcW1_t = nc.dram_tensor("fcW1", [HID, HMLP], F32, kind="ExternalInput")
    fcb1_t = nc.dram_tensor("fcb1", [1, HMLP], F32, kind="ExternalInput")
    fcW2_t = nc.dram_tensor("fcW2", [HMLP, NCL], F32, kind="ExternalInput")
    fcb2_t = nc.dram_tensor("fcb2", [1, NCL], F32, kind="ExternalInput")
    out_t = nc.dram_tensor("out", [NG, NCL], F32, kind="ExternalOutput")

    KB1 = CIN // 128    # K blocks layer1 (1)
    KB2 = HID // 128    # K blocks layer2 (2)
    KBM = HID // 128    # fc1 K blocks (2)

    with tile.TileContext(nc) as tc:
        with (
            tc.tile_pool(name="const", bufs=1) as cpool,
            tc.tile_pool(name="gath", bufs=3) as gpool,
            tc.tile_pool(name="work", bufs=3) as wpool,
            tc.tile_pool(name="outp", bufs=3) as opool,
            tc.tile_pool(name="tp_ps", bufs=2, space="PSUM") as tp_ps,
            tc.tile_pool(name="mm_ps", bufs=2, space="PSUM") as mm_ps,
            tc.tile_pool(name="dram", bufs=1, space="DRAM") as dr,
        ):
            # ---- constants / weights to SBUF
            ident = cpool.tile([128, 128], F32)
            make_identity(nc, ident[:])
            ones = cpool.tile([1, 128], F32)
            nc.vector.memset(ones[:], 1.0)
            negbig = cpool.tile([128, HID], F32)
            nc.vector.memset(negbig[:], NEG_BIG)

            W1_sb = cpool.tile([128, KB1, HID], F32)
            for k in range(KB1):
                nc.sync.dma_start(out=W1_sb[:, k, :],
                                  in_=W1_t[k * 128:(k + 1) * 128, :])
            W2_sb = cpool.tile([128, KB2, HID], F32)
            for k in range(KB2):
                nc.sync.dma_start(out=W2_sb[:, k, :],
                                  in_=W2_t[k * 128:(k + 1) * 128, :])
            fcW1_sb = cpool.tile([128, KBM, HMLP], F32)
            for k in range(KBM):
                nc.sync.dma_start(out=fcW1_sb[:, k, :],
                                  in_=fcW1_t[k * 128:(k + 1) * 128, :])
            fcW2_sb = cpool.tile([128, NCL], F32)
            nc.sync.dma_start(out=fcW2_sb[:], in_=fcW2_t[:, :])
            b1_sb = cpool.tile([1, HID], F32)
            nc.sync.dma_start(out=b1_sb[:], in_=b1_t[:, :])
            b2_sb = cpool.tile([1, HID], F32)
            nc.sync.dma_start(out=b2_sb[:], in_=b2_t[:, :])
            fcb1_sb = cpool.tile([1, HMLP], F32)
            nc.sync.dma_start(out=fcb1_sb[:], in_=fcb1_t[:, :])
            fcb2_sb = cpool.tile([1, NCL], F32)
            nc.sync.dma_start(out=fcb2_sb[:], in_=fcb2_t[:, :])

            idx1_sb = cpool.tile([128, n_chunk_cols * 8], I16)
            nc.sync.dma_start(out=idx1_sb[:], in_=idx1_t[:, :])
            idx2_sb = cpool.tile([128, n_chunk_cols * 8], I16)
            nc.sync.dma_start(out=idx2_sb[:], in_=idx2_t[:, :])
            dinv1_sb = cpool.tile([128, sumJ], F32)
            nc.sync.dma_start(out=dinv1_sb[:], in_=dinv1_t[:, :])
            dinvd_sb = cpool.tile([128, T], F32)
            nc.sync.dma_start(out=dinvd_sb[:], in_=dinvd_t[:, :])
            idxp_sb = cpool.tile([128, n_pool_cols * 8], I16)
            nc.sync.dma_start(out=idxp_sb[:], in_=idxp_t[:, :])
            scat_sb = cpool.tile([128, 1], I32)
            nc.sync.dma_start(out=scat_sb[:], in_=scat_t[:, :])

            # ---- internal DRAM
            h1_shard = dr.tile([LV, HID], F32)
            h1_table = dr.tile([NP, HID], F32, addr_space="Shared")
            h2_local = dr.tile([LV + 1, HID], F32)
            pool_scat = dr.tile([NG + 1, HID], F32)
            pool_red = dr.tile([NG, HID], F32, addr_space="Shared")

            # init h2 pad row + pool_scat table to NEG_BIG
            nc.sync.dma_start(out=h2_local[LV:LV + 1, :], in_=negbig[0:1, :])
            for i in range(cdiv(NG + 1, 128)):
                r0 = i * 128
                r1 = min(r0 + 128, NG + 1)
                nc.sync.dma_start(out=pool_scat[r0:r1, :],
                                  in_=negbig[0:r1 - r0, :])

            # ---------------- layer helper
            def gcn_layer(layer):
                if layer == 1:
                    C = CIN
                    idx_sb = idx1_sb
                    table_ap = x_t[:, :]
                    KB, W_sb, b_sb = KB1, W1_sb, b1_sb
                    dst_dram = h1_shard
                else:
                    C = HID
                    idx_sb = idx2_sb
                    table_ap = h1_table[:, :]
                    KB, W_sb, b_sb = KB2, W2_sb, b2_sb
                    dst_dram = h2_local
                base = m["BASE1"] if layer == 1 else m["BASE2"]

                icol = 0   # idx column offset (units of 8 int16 per slot)
                slot = 0   # global slot offset (dinv1 columns)
                gtag = "g1" if layer == 1 else "g2"
                for t in range(T):
                    acc = wpool.tile([128, HID], F32, tag="acc")
                    first = True
                    for w in chunks[t]:
                        g = gpool.tile([128, 8, C], F32, tag=gtag)
                        nc.gpsimd.dma_gather(
                            g[:, 0:w + 1, 0:C],
                            table_ap[base:, :],
                            idx_sb[:, icol * 8:(icol + w + 1) * 8],
                            (w + 1) * 128, (w + 1) * 128, C)
                        for j in range(w):
                            if layer == 1:
                                if first:
                                    nc.vector.tensor_scalar(
                                        out=acc[:, 0:C], in0=g[:, j, 0:C],
                                        scalar1=dinv1_sb[:, slot:slot + 1],
                                        scalar2=None,
                                        op0=mybir.AluOpType.mult)
                                else:
                                    nc.vector.scalar_tensor_tensor(
                                        out=acc[:, 0:C], in0=g[:, j, 0:C],
                                        scalar=dinv1_sb[:, slot:slot + 1],
                                        in1=acc[:, 0:C],
                                        op0=mybir.AluOpType.mult,
                                        op1=mybir.AluOpType.add)
                            else:
                                if first:
                                    nc.vector.tensor_copy(out=acc[:, 0:C],
                                                          in_=g[:, j, 0:C])
                                else:
                                    nc.vector.tensor_add(
                                        out=acc[:, 0:C], in0=acc[:, 0:C],
                                        in1=g[:, j, 0:C])
                            first = False
                            slot += 1
                        icol += w + 1

                    # dst-side dinv scaling
                    nc.vector.tensor_scalar_mul(
                        out=acc[:, 0:C], in0=acc[:, 0:C],
                        scalar1=dinvd_sb[:, t:t + 1])

                    # transpose -> lhsT blocks
                    accT = wpool.tile([128, KB, 128], F32, tag="accT")
                    for k in range(KB):
                        tps = tp_ps.tile([128, 128], F32, tag="tp")
                        nc.tensor.transpose(out=tps[:],
                                            in_=acc[:, k * 128:(k + 1) * 128],
                                            identity=ident[:])
                        nc.vector.tensor_copy(out=accT[:, k, :], in_=tps[:])

                    # matmul: bias + sum_k accT_k.T @ W_k
                    mm = mm_ps.tile([128, HID], F32, tag="mm")
                    nc.tensor.matmul(out=mm[:], lhsT=ones[0:1, :],
                                     rhs=b_sb[0:1, :], start=True, stop=False)
                    for k in range(KB):
                        nc.tensor.matmul(out=mm[:], lhsT=accT[:, k, :],
                                         rhs=W_sb[:, k, :],
                                         start=False, stop=(k == KB - 1))

                    h = opool.tile([128, HID], F32, tag="h")
                    if layer == 1:
                        # h1' = relu(dinv * (aggW + b)) = dinv * relu(aggW+b)
                        nc.scalar.activation(
                            out=h[:], in_=mm[:],
                            func=mybir.ActivationFunctionType.Relu,
                            scale=dinvd_sb[:, t:t + 1])
                    else:
                        nc.scalar.activation(
                            out=h[:], in_=mm[:],
                            func=mybir.ActivationFunctionType.Relu)
                    nc.sync.dma_start(
                        out=dst_dram[t * 128:(t + 1) * 128, :], in_=h[:])

            # ---------------- layer 1 + allgather
            gcn_layer(1)
            if stage >= 2:
              nc.gpsimd.collective_compute(
                "AllGather", mybir.AluOpType.bypass,
                replica_groups=[list(range(NC))],
                ins=[h1_shard[:, :]], outs=[h1_table[:, :]])

            # ---------------- layer 2
            if stage >= 3:
              gcn_layer(2)

            # ---------------- pooling
            pooled = wpool.tile([128, HID], F32, tag="pooled")
            if stage >= 4:
             pcol = 0
             firstp = True
             for w in pool_chunks:
                gp = gpool.tile([128, 8, HID], F32, tag="g2")
                nc.gpsimd.dma_gather(
                    gp[:, 0:w + 1, :],
                    h2_local[0:, :],
                    idxp_sb[:, pcol * 8:(pcol + w + 1) * 8],
                    (w + 1) * 128, (w + 1) * 128, HID)
                # strided view [128, HID, w]: reduce innermost (w)
                red = wpool.tile([128, HID], F32, tag="red")
                gv = gp[:, 0:w, :].rearrange("p j c -> p c j")
                nc.vector.tensor_reduce(out=red[:], in_=gv,
                                        axis=mybir.AxisListType.X,
                                        op=mybir.AluOpType.max)
                if firstp:
                    nc.vector.tensor_copy(out=pooled[:], in_=red[:])
                else:
                    nc.vector.tensor_max(out=pooled[:], in0=pooled[:],
                                         in1=red[:])
                firstp = False
                pcol += w + 1
            if stage < 4:
                nc.vector.memset(pooled[:], 0.0)
            if stage < 5:
                nc.sync.dma_start(out=out_t[0:min(128, m['n_graphs']), 0:NCL],
                                  in_=pooled[0:min(128, m['n_graphs']), 0:NCL])
            if stage >= 5:
              nc.gpsimd.indirect_dma_start(
                out=pool_scat[:, :],
                out_offset=bass.IndirectOffsetOnAxis(ap=scat_sb[:, 0:1],
                                                     axis=0),
                in_=pooled[:],
                in_offset=None)
              nc.gpsimd.collective_compute(
                "AllReduce", mybir.AluOpType.max,
                replica_groups=[list(range(NC))],
                ins=[pool_scat[0:NG, :]], outs=[pool_red[:, :]])

            # ---------------- MLP + log_softmax (replicated)
            if stage < 5:
                nc.compile_marker = None
            gT = wpool.tile([128, KBM, NGT * 128], F32, tag="gT")
            for i in range(NGT) if stage >= 5 else []:
                gtile = wpool.tile([128, HID], F32, tag="gtile")
                r0, r1 = i * 128, min((i + 1) * 128, NG)
                if r1 - r0 < 128:
                    nc.vector.memset(gtile[:], 0.0)
                nc.sync.dma_start(out=gtile[0:r1 - r0, :],
                                  in_=pool_red[r0:r1, :])
                for k in range(KBM):
                    tps = tp_ps.tile([128, 128], F32, tag="tp")
                    nc.tensor.transpose(out=tps[:],
                                        in_=gtile[:, k * 128:(k + 1) * 128],
                                        identity=ident[:])
                    nc.vector.tensor_copy(out=gT[:, k, i * 128:(i + 1) * 128],
                                          in_=tps[:])
            o1T = wpool.tile([128, NGT * 128], F32, tag="o1T")
            for i in range(NGT) if stage >= 5 else []:
                mm1 = mm_ps.tile([128, HMLP], F32, tag="mm")
                nc.tensor.matmul(out=mm1[:], lhsT=ones[0:1, :],
                                 rhs=fcb1_sb[0:1, :], start=True, stop=False)
                for k in range(KBM):
                    nc.tensor.matmul(out=mm1[:],
                                     lhsT=gT[:, k, i * 128:(i + 1) * 128],
                                     rhs=fcW1_sb[:, k, :],
                                     start=False, stop=(k == KBM - 1))
                o1 = wpool.tile([128, HMLP], F32, tag="o1")
                nc.scalar.activation(out=o1[:], in_=mm1[:],
                                     func=mybir.ActivationFunctionType.Relu)
                tps = tp_ps.tile([128, 128], F32, tag="tp")
                nc.tensor.transpose(out=tps[0:HMLP, :], in_=o1[:],
                                    identity=ident[:])
                nc.vector.tensor_copy(out=o1T[0:HMLP, i * 128:(i + 1) * 128],
                                      in_=tps[0:HMLP, :])
            for i in range(NGT) if stage >= 5 else []:
                mm2 = mm_ps.tile([128, NCL], F32, tag="mm2")
                nc.tensor.matmul(out=mm2[:], lhsT=ones[0:1, :],
                                 rhs=fcb2_sb[0:1, :], start=True, stop=False)
                nc.tensor.matmul(out=mm2[:],
                                 lhsT=o1T[0:HMLP, i * 128:(i + 1) * 128],
                                 rhs=fcW2_sb[0:HMLP, :],
                                 start=False, stop=True)
                # log_softmax rows
                mx = wpool.tile([128, 1], F32, tag="mx")
                nc.vector.tensor_reduce(out=mx[:], in_=mm2[:],
                                        axis=mybir.AxisListType.X,
                                        op=mybir.AluOpType.max)
                tsh = wpool.tile([128, NCL], F32, tag="tsh")
                nc.vector.tensor_scalar(
                    out=tsh[:], in0=mm2[:], scalar1=mx[:, 0:1], scalar2=None,
                    op0=mybir.AluOpType.subtract)
                ex = wpool.tile([128, NCL], F32, tag="ex")
                nc.scalar.activation(out=ex[:], in_=tsh[:],
                                     func=mybir.ActivationFunctionType.Exp)
                sm = wpool.tile([128, 1], F32, tag="sm")
                nc.vector.tensor_reduce(out=sm[:], in_=ex[:],
                                        axis=mybir.AxisListType.X,
                                        op=mybir.AluOpType.add)
                ls = wpool.tile([128, 1], F32, tag="ls")
                nc.scalar.activation(out=ls[:], in_=sm[:],
                                     func=mybir.ActivationFunctionType.Ln)
                oo = opool.tile([128, NCL], F32, tag="oo")
                nc.vector.tensor_scalar(
                    out=oo[:], in0=tsh[:], scalar1=ls[:, 0:1], scalar2=None,
                    op0=mybir.AluOpType.subtract)
                r0, r1 = i * 128, min((i + 1) * 128, NG)
                nc.sync.dma_start(out=out_t[r0:r1, :], in_=oo[0:r1 - r0, :])

    nc.compile()
    return nc


# ---------------------------------------------------------------- entry

def _ensure_ntff_hook():
    """Install the axon NTFF profile hook if the image's antenv lacks it.

    Dev-only (trace=True): lets run_bass_kernel_spmd return exec_time_ns.
    """
    import sys as _sys
    import types as _types
    try:
        from antenv.axon_hooks import get_axon_ntff_profile_hook  # noqa
        return
    except ImportError:
        pass
    try:
        _sys.path.insert(0, "/root/.axon_site")
        from trn_agent_boot.trn_boot import _ntff_profile_via_ctypes
        hook = _ntff_profile_via_ctypes("/opt/axon/libaxon_pjrt.so")
        mod = _types.ModuleType("antenv.axon_hooks")
        mod._hook = hook
        mod.get_axon_ntff_profile_hook = lambda: mod._hook
        mod.set_axon_ntff_profile_hook = lambda h: setattr(mod, "_hook", h)
        _sys.modules["antenv.axon_hooks"] = mod
        # artifact upload needs a bucket; degrade to no-op on failure
        _orig_upload = bass_utils.upload_artifacts

        def _safe_upload(tmpdir):
            try:
                return _orig_upload(tmpdir)
            except Exception:
                return tmpdir
        bass_utils.upload_artifacts = _safe_upload
    except Exception:
        pass


def kernel(x, edge_index, batch, W1, b1, W2, b2, fcW1, fcb1, fcW2, fcb2,
           trace=False):
    if trace:
        _ensure_ntff_hook()
    x = np.asarray(x, np.float32)
    edge_index = np.asarray(edge_index, np.int64)
    batch = np.asarray(batch, np.int64)
    NG = 512
    meta, per_core = prep(x, edge_index, batch, NG)

    nc = build(meta, CIN=x.shape[1], HID=W1.shape[1], HMLP=fcW1.shape[1],
               NCL=fcW2.shape[1])

    shared = dict(
        x=x,
        W1=np.asarray(W1, np.float32), b1=np.asarray(b1, np.float32)[None, :],
        W2=np.asarray(W2, np.float32), b2=np.asarray(b2, np.float32)[None, :],
        fcW1=np.asarray(fcW1, np.float32),
        fcb1=np.asarray(fcb1, np.float32)[None, :],
        fcW2=np.asarray(fcW2, np.float32),
        fcb2=np.asarray(fcb2, np.float32)[None, :],
    )
    in_maps = []
    for c in range(meta["NC"]):
        d = dict(shared)
        pc = per_core[c]
        d["idx1"] = pc["idx1"]
        d["idx2"] = pc["idx2"]
        d["dinv1"] = pc["dinv1"]
        d["dinv_dst"] = pc["dinv_dst"]
        d["idxp"] = pc["idxp"]
        d["scat_g"] = np.tile(pc["scat_g"], (1, 1))
        in_maps.append(d)

    res = bass_utils.run_bass_kernel_spmd(
        nc, in_maps, core_ids=list(range(meta["NC"])), trace=trace)
    out = res.results[0]["out"]
    kernel.last_exec_time_ns = res.exec_time_ns
    return out


kernel.last_exec_time_ns = None



# revision 4
# speedup vs baseline: 1.1184x; 1.1184x over previous
"""GCN (2-layer GCNConv + global max pool + MLP + log_softmax) on 8 trn2 cores.

Strategy (sharding_hint: partition nodes + incident edges, replicate weights):
  - Nodes are partitioned 6250/core (+22 pad nodes/core -> 6272 = 49 tiles of
    128). Within each core, nodes are sorted by degree (desc) so that the
    per-tile padded gather width J_t ~= the true degree.
  - Edges are grouped by dst; each core owns edges into its nodes. For each
    128-node tile the messages are fetched with dma_gather (int16 signed
    indices relative to a mid-table base row cover all 50176/50000 rows),
    giving [128 nodes, J, C] tiles which are reduced on DVE.
  - GCN normalization: agg = D^-1/2 (A+I) D^-1/2 h. Layer-1 folds
    dinv[src] into the DVE accumulate; the produced h1 is pre-scaled by
    dinv (h1' = dinv * relu(...)), so layer-2 accumulation is plain adds.
  - h1' shards are AllGathered into a replicated table; layer-2 gathers
    from it. Max pooling is a dma_gather per graph-partition from the local
    h2 shard + DVE max-reduce, scattered into a [513,256] table (indirect
    scatter handles per-core graph offsets), AllReduce(max), then the small
    MLP + log_softmax run replicated on every core.
"""

import numpy as np

import concourse.bass as bass
import concourse.bacc as bacc
import concourse.tile as tile
import concourse.mybir as mybir
from concourse import bass_utils
from concourse.masks import make_identity
from concourse._compat import cdiv

F32 = mybir.dt.float32
I16 = mybir.dt.int16
I32 = mybir.dt.int32

NEG_BIG = -1.0e38


# ---------------------------------------------------------------- host prep

def _wrap_idx(flat):
    """j-major flat int16 idx list [n] -> wrapped SBUF layout [128, n//16].

    dma_gather consumes idx i from wrapped[i % 16, i // 16]; the 16-row
    pattern is replicated to all 128 partitions.
    """
    n = len(flat)
    assert n % 128 == 0
    w = np.zeros((16, n // 16), np.int16)
    w[np.arange(n) % 16, np.arange(n) // 16] = flat
    return np.tile(w, (8, 1))


def prep(x, edge_index, batch, n_graphs, n_cores=8, j_cap=6, pool_cap=6,
         mid_base=True):
    """All index-space preprocessing. Returns (meta, per-core arrays)."""
    N = x.shape[0]
    NR = N // n_cores                      # real nodes per core
    LV = int(cdiv(NR, 128)) * 128          # padded nodes per core
    T = LV // 128                          # tiles per core
    NP = LV * n_cores                      # padded total
    BASE1 = N // 2 if mid_base else 0      # x-table base row
    BASE2 = NP // 2 if mid_base else 0     # h1-table base row
    assert max(N - BASE1, BASE1, NP - BASE2, BASE2, NR + 1) <= 32767

    src = np.concatenate([edge_index[0], np.arange(N, dtype=np.int64)])
    dst = np.concatenate([edge_index[1], np.arange(N, dtype=np.int64)])
    deg = np.bincount(dst, minlength=N).astype(np.int64)
    dinv = np.zeros(N, np.float32)
    nz = deg > 0
    dinv[nz] = 1.0 / np.sqrt(np.maximum(deg[nz], 1).astype(np.float32))

    # group edges by dst
    order = np.argsort(dst, kind="stable")
    src_s = src[order]
    starts = np.searchsorted(dst[order], np.arange(N))
    ends = np.searchsorted(dst[order], np.arange(N) + 1)

    # per-core degree-sorted permutation; perm[c][l] = orig id, -1 = pad
    perm = np.full((n_cores, LV), -1, np.int64)
    for c in range(n_cores):
        lo = NR * c
        perm[c, :NR] = np.argsort(-deg[lo:lo + NR], kind="stable") + lo
    perm_row = np.full(N, -1, np.int64)   # orig id -> permuted global row
    for c in range(n_cores):
        perm_row[perm[c, :NR]] = LV * c + np.arange(NR)

    # J_t per tile (max over cores), split into gather chunks of width <=j_cap
    Jt = np.zeros(T, np.int64)
    for c in range(n_cores):
        for t in range(T):
            ids = perm[c, t * 128:(t + 1) * 128]
            ids = ids[ids >= 0]
            if len(ids):
                Jt[t] = max(Jt[t], deg[ids].max())
    Jt = np.maximum(Jt, 1)
    chunks = [[j_cap] * (int(j) // j_cap) + ([int(j) % j_cap] if j % j_cap else [])
              for j in Jt]
    sumJ = int(Jt.sum())

    # pad rows: any pad node's permuted row (h1' there is forced to 0);
    # use the globally-last pad row.
    pad_row2 = NP - 1 if NP > N else None
    assert pad_row2 is not None, "need at least one pad node for L2 padding"

    # per-core slot tables
    per_core = []
    for c in range(n_cores):
        idx1 = np.zeros((sumJ * 128,), np.int16)
        idx2 = np.zeros((sumJ * 128,), np.int16)
        dinv1 = np.zeros((128, sumJ), np.float32)
        dinv_dst = np.zeros((128, T), np.float32)
        off = 0
        for t in range(T):
            J = int(Jt[t])
            for p in range(128):
                n = perm[c, t * 128 + p]
                if n >= 0:
                    dinv_dst[p, t] = dinv[n]
                    ss = src_s[starts[n]:ends[n]]
                    nj = len(ss)
                    sl = (off + np.arange(nj)) * 128 + p
                    idx1[sl] = (ss - BASE1).astype(np.int16)
                    idx2[sl] = (perm_row[ss] - BASE2).astype(np.int16)
                    dinv1[p, off:off + nj] = dinv[ss]
                else:
                    nj = 0
                # pad slots
                if nj < J:
                    sl = (off + np.arange(nj, J)) * 128 + p
                    idx1[sl] = 0            # dinv1 = 0 neutralizes
                    idx2[sl] = pad_row2 - BASE2   # zero row
            off += J
        assert off == sumJ

        # wrapped layout per gather chunk; each chunk gets one trailing
        # all-pad block: the gather's final descriptor flakily skips its
        # data write, so the last 128 slots are sacrificial and never read.
        pad_blk = np.zeros(128, np.int16)
        w1 = []
        w2 = []
        off = 0
        for t in range(T):
            for w in chunks[t]:
                blk = slice(off * 128, (off + w) * 128)
                w1.append(_wrap_idx(np.concatenate([idx1[blk], pad_blk])))
                w2.append(_wrap_idx(np.concatenate([idx2[blk], pad_blk])))
                off += w
        idx1_w = np.concatenate(w1, axis=1)
        idx2_w = np.concatenate(w2, axis=1)
        per_core.append(dict(idx1=idx1_w, idx2=idx2_w, dinv1=dinv1,
                             dinv_dst=dinv_dst))

    # pooling: per-core graph ranges + member lists (permuted-local rows)
    glo = np.zeros(n_cores, np.int64)
    Gc = np.zeros(n_cores, np.int64)
    for c in range(n_cores):
        b = batch[NR * c:NR * (c + 1)]
        glo[c] = b.min()
        Gc[c] = b.max() - b.min() + 1
    G_max = int(Gc.max())
    assert G_max <= 128
    # member lists
    members = []   # [core][local graph] -> list of local permuted rows
    for c in range(n_cores):
        b = batch[NR * c:NR * (c + 1)]
        loc = [[] for _ in range(G_max)]
        inv_l = np.empty(NR, np.int64)
        inv_l[perm[c, :NR] - NR * c] = np.arange(NR)
        for i in range(NR):
            loc[int(b[i] - glo[c])].append(int(inv_l[i]))
        members.append(loc)
    Jp = max(max(len(m) for m in loc) for loc in members)
    pool_chunks = [pool_cap] * (Jp // pool_cap) + \
        ([Jp % pool_cap] if Jp % pool_cap else [])
    PAD_POOL = LV  # row LV of h2_local = NEG_BIG
    for c in range(n_cores):
        flat = np.full((len(pool_chunks) and sum(pool_chunks)) * 128, PAD_POOL,
                       np.int16)
        loc = members[c]
        for p in range(128):
            mem = loc[p] if p < G_max else []
            for j, r in enumerate(mem):
                flat[j * 128 + p] = r
        w = []
        off = 0
        pad_blk0 = np.zeros(128, np.int16)
        for wdt in pool_chunks:
            blk = slice(off * 128, (off + wdt) * 128)
            w.append(_wrap_idx(np.concatenate([flat[blk], pad_blk0])))
            off += wdt
        per_core[c]["idxp"] = np.concatenate(w, axis=1)
        scat = np.full(128, n_graphs, np.int64)
        scat[:int(Gc[c])] = glo[c] + np.arange(int(Gc[c]))
        per_core[c]["scat_g"] = scat.astype(np.int32)[:, None]

    meta = dict(N=N, NP=NP, LV=LV, T=T, NC=n_cores, BASE1=BASE1, BASE2=BASE2,
                chunks=chunks, sumJ=sumJ, pool_chunks=pool_chunks,
                n_graphs=n_graphs)
    return meta, per_core


# ---------------------------------------------------------------- bass build

def build(meta, CIN, HID, HMLP, NCL, stage=5, n_queues=4):
    """Build the SPMD Bass program. All per-core variation flows via inputs."""
    m = meta
    T, NC = m["T"], m["NC"]
    N, NP, LV = m["N"], m["NP"], m["LV"]
    chunks, pool_chunks = m["chunks"], m["pool_chunks"]
    sumJ = m["sumJ"]
    NG = m["n_graphs"]
    NGT = cdiv(NG, 128)          # pooled tiles (4)
    n_chunk_cols = sum(sum(w + 1 for w in cl) for cl in chunks)
    n_pool_cols = sum(w + 1 for w in pool_chunks)

    nc = bacc.Bacc("TRN2", target_bir_lowering=False, debug=False,
                   num_devices=NC, num_swdge_queues=n_queues)
    qctr = [0]

    def next_q():
        q = qctr[0] % n_queues
        qctr[0] += 1
        return q
    dt = mybir.dt

    # ---- inputs
    x_t = nc.dram_tensor("x", [N, CIN], F32, kind="ExternalInput")
    idx1_t = nc.dram_tensor("idx1", [128, n_chunk_cols * 8], I16,
                            kind="ExternalInput")
    idx2_t = nc.dram_tensor("idx2", [128, n_chunk_cols * 8], I16,
                            kind="ExternalInput")
    dinv1_t = nc.dram_tensor("dinv1", [128, sumJ], F32, kind="ExternalInput")
    dinvd_t = nc.dram_tensor("dinv_dst", [128, T], F32, kind="ExternalInput")
    idxp_t = nc.dram_tensor("idxp", [128, n_pool_cols * 8], I16,
                            kind="ExternalInput")
    scat_t = nc.dram_tensor("scat_g", [128, 1], I32, kind="ExternalInput")
    W1_t = nc.dram_tensor("W1", [CIN, HID], F32, kind="ExternalInput")
    b1_t = nc.dram_tensor("b1", [1, HID], F32, kind="ExternalInput")
    W2_t = nc.dram_tensor("W2", [HID, HID], F32, kind="ExternalInput")
    b2_t = nc.dram_tensor("b2", [1, HID], F32, kind="ExternalInput")
    fcW1_t = nc.dram_tensor("fcW1", [HID, HMLP], F32, kind="ExternalInput")
    fcb1_t = nc.dram_tensor("fcb1", [1, HMLP], F32, kind="ExternalInput")
    fcW2_t = nc.dram_tensor("fcW2", [HMLP, NCL], F32, kind="ExternalInput")
    fcb2_t = nc.dram_tensor("fcb2", [1, NCL], F32, kind="ExternalInput")
    out_t = nc.dram_tensor("out", [NG, NCL], F32, kind="ExternalOutput")

    KB1 = CIN // 128    # K blocks layer1 (1)
    KB2 = HID // 128    # K blocks layer2 (2)
    KBM = HID // 128    # fc1 K blocks (2)

    with tile.TileContext(nc) as tc:
        with (
            tc.tile_pool(name="const", bufs=1) as cpool,
            tc.tile_pool(name="gath", bufs=3) as gpool,
            tc.tile_pool(name="work", bufs=3) as wpool,
            tc.tile_pool(name="outp", bufs=3) as opool,
            tc.tile_pool(name="tp_ps", bufs=2, space="PSUM") as tp_ps,
            tc.tile_pool(name="mm_ps", bufs=2, space="PSUM") as mm_ps,
            tc.tile_pool(name="dram", bufs=1, space="DRAM") as dr,
        ):
            # ---- constants / weights to SBUF
            ident = cpool.tile([128, 128], F32)
            make_identity(nc, ident[:])
            ones = cpool.tile([1, 128], F32)
            nc.vector.memset(ones[:], 1.0)
            negbig = cpool.tile([128, HID], F32)
            nc.vector.memset(negbig[:], NEG_BIG)

            W1_sb = cpool.tile([128, KB1, HID], F32)
            for k in range(KB1):
                nc.sync.dma_start(out=W1_sb[:, k, :],
                                  in_=W1_t[k * 128:(k + 1) * 128, :])
            W2_sb = cpool.tile([128, KB2, HID], F32)
            for k in range(KB2):
                nc.sync.dma_start(out=W2_sb[:, k, :],
                                  in_=W2_t[k * 128:(k + 1) * 128, :])
            fcW1_sb = cpool.tile([128, KBM, HMLP], F32)
            for k in range(KBM):
                nc.sync.dma_start(out=fcW1_sb[:, k, :],
                                  in_=fcW1_t[k * 128:(k + 1) * 128, :])
            fcW2_sb = cpool.tile([128, NCL], F32)
            nc.sync.dma_start(out=fcW2_sb[:], in_=fcW2_t[:, :])
            b1_sb = cpool.tile([1, HID], F32)
            nc.sync.dma_start(out=b1_sb[:], in_=b1_t[:, :])
            b2_sb = cpool.tile([1, HID], F32)
            nc.sync.dma_start(out=b2_sb[:], in_=b2_t[:, :])
            fcb1_sb = cpool.tile([1, HMLP], F32)
            nc.sync.dma_start(out=fcb1_sb[:], in_=fcb1_t[:, :])
            fcb2_sb = cpool.tile([1, NCL], F32)
            nc.sync.dma_start(out=fcb2_sb[:], in_=fcb2_t[:, :])

            idx1_sb = cpool.tile([128, n_chunk_cols * 8], I16)
            nc.sync.dma_start(out=idx1_sb[:], in_=idx1_t[:, :])
            idx2_sb = cpool.tile([128, n_chunk_cols * 8], I16)
            nc.sync.dma_start(out=idx2_sb[:], in_=idx2_t[:, :])
            dinv1_sb = cpool.tile([128, sumJ], F32)
            nc.sync.dma_start(out=dinv1_sb[:], in_=dinv1_t[:, :])
            dinvd_sb = cpool.tile([128, T], F32)
            nc.sync.dma_start(out=dinvd_sb[:], in_=dinvd_t[:, :])
            idxp_sb = cpool.tile([128, n_pool_cols * 8], I16)
            nc.sync.dma_start(out=idxp_sb[:], in_=idxp_t[:, :])
            scat_sb = cpool.tile([128, 1], I32)
            nc.sync.dma_start(out=scat_sb[:], in_=scat_t[:, :])

            # ---- internal DRAM
            h1_shard = dr.tile([LV, HID], F32)
            h1_table = dr.tile([NP, HID], F32, addr_space="Shared")
            h2_local = dr.tile([LV + 1, HID], F32)
            pool_scat = dr.tile([NG + 1, HID], F32)
            pool_red = dr.tile([NG, HID], F32, addr_space="Shared")

            # init h2 pad row + pool_scat table to NEG_BIG
            nc.sync.dma_start(out=h2_local[LV:LV + 1, :], in_=negbig[0:1, :])
            for i in range(cdiv(NG + 1, 128)):
                r0 = i * 128
                r1 = min(r0 + 128, NG + 1)
                nc.sync.dma_start(out=pool_scat[r0:r1, :],
                                  in_=negbig[0:r1 - r0, :])

            # ---------------- layer helper
            def gcn_layer(layer):
                if layer == 1:
                    C = CIN
                    idx_sb = idx1_sb
                    table_ap = x_t[:, :]
                    KB, W_sb, b_sb = KB1, W1_sb, b1_sb
                    dst_dram = h1_shard
                else:
                    C = HID
                    idx_sb = idx2_sb
                    table_ap = h1_table[:, :]
                    KB, W_sb, b_sb = KB2, W2_sb, b2_sb
                    dst_dram = h2_local
                base = m["BASE1"] if layer == 1 else m["BASE2"]

                icol = 0   # idx column offset (units of 8 int16 per slot)
                slot = 0   # global slot offset (dinv1 columns)
                gtag = "g1" if layer == 1 else "g2"
                for t in range(T):
                    acc = wpool.tile([128, HID], F32, tag="acc")
                    first = True
                    for w in chunks[t]:
                        g = gpool.tile([128, 8, C], F32, tag=gtag)
                        nc.gpsimd.dma_gather(
                            g[:, 0:w + 1, 0:C],
                            table_ap[base:, :],
                            idx_sb[:, icol * 8:(icol + w + 1) * 8],
                            (w + 1) * 128, (w + 1) * 128, C,
                            queue_num=next_q())
                        for j in range(w):
                            if layer == 1:
                                if first:
                                    nc.vector.tensor_scalar(
                                        out=acc[:, 0:C], in0=g[:, j, 0:C],
                                        scalar1=dinv1_sb[:, slot:slot + 1],
                                        scalar2=None,
                                        op0=mybir.AluOpType.mult)
                                else:
                                    nc.vector.scalar_tensor_tensor(
                                        out=acc[:, 0:C], in0=g[:, j, 0:C],
                                        scalar=dinv1_sb[:, slot:slot + 1],
                                        in1=acc[:, 0:C],
                                        op0=mybir.AluOpType.mult,
                                        op1=mybir.AluOpType.add)
                            else:
                                if first:
                                    nc.vector.tensor_copy(out=acc[:, 0:C],
                                                          in_=g[:, j, 0:C])
                                else:
                                    nc.vector.tensor_add(
                                        out=acc[:, 0:C], in0=acc[:, 0:C],
                                        in1=g[:, j, 0:C])
                            first = False
                            slot += 1
                        icol += w + 1

                    # dst-side dinv scaling
                    nc.vector.tensor_scalar_mul(
                        out=acc[:, 0:C], in0=acc[:, 0:C],
                        scalar1=dinvd_sb[:, t:t + 1])

                    # transpose -> lhsT blocks
                    accT = wpool.tile([128, KB, 128], F32, tag="accT")
                    for k in range(KB):
                        tps = tp_ps.tile([128, 128], F32, tag="tp")
                        nc.tensor.transpose(out=tps[:],
                                            in_=acc[:, k * 128:(k + 1) * 128],
                                            identity=ident[:])
                        nc.vector.tensor_copy(out=accT[:, k, :], in_=tps[:])

                    # matmul: bias + sum_k accT_k.T @ W_k
                    mm = mm_ps.tile([128, HID], F32, tag="mm")
                    nc.tensor.matmul(out=mm[:], lhsT=ones[0:1, :],
                                     rhs=b_sb[0:1, :], start=True, stop=False)
                    for k in range(KB):
                        nc.tensor.matmul(out=mm[:], lhsT=accT[:, k, :],
                                         rhs=W_sb[:, k, :],
                                         start=False, stop=(k == KB - 1))

                    h = opool.tile([128, HID], F32, tag="h")
                    if layer == 1:
                        # h1' = relu(dinv * (aggW + b)) = dinv * relu(aggW+b)
                        nc.scalar.activation(
                            out=h[:], in_=mm[:],
                            func=mybir.ActivationFunctionType.Relu,
                            scale=dinvd_sb[:, t:t + 1])
                    else:
                        nc.scalar.activation(
                            out=h[:], in_=mm[:],
                            func=mybir.ActivationFunctionType.Relu)
                    nc.sync.dma_start(
                        out=dst_dram[t * 128:(t + 1) * 128, :], in_=h[:])

            # ---------------- layer 1 + allgather
            gcn_layer(1)
            if stage >= 2:
              nc.gpsimd.collective_compute(
                "AllGather", mybir.AluOpType.bypass,
                replica_groups=[list(range(NC))],
                ins=[h1_shard[:, :]], outs=[h1_table[:, :]])

            # ---------------- layer 2
            if stage >= 3:
              gcn_layer(2)

            # ---------------- pooling
            pooled = wpool.tile([128, HID], F32, tag="pooled")
            if stage >= 4:
             pcol = 0
             firstp = True
             for w in pool_chunks:
                gp = gpool.tile([128, 8, HID], F32, tag="g2")
                nc.gpsimd.dma_gather(
                    gp[:, 0:w + 1, :],
                    h2_local[0:, :],
                    idxp_sb[:, pcol * 8:(pcol + w + 1) * 8],
                    (w + 1) * 128, (w + 1) * 128, HID,
                    queue_num=next_q())
                # strided view [128, HID, w]: reduce innermost (w)
                red = wpool.tile([128, HID], F32, tag="red")
                gv = gp[:, 0:w, :].rearrange("p j c -> p c j")
                nc.vector.tensor_reduce(out=red[:], in_=gv,
                                        axis=mybir.AxisListType.X,
                                        op=mybir.AluOpType.max)
                if firstp:
                    nc.vector.tensor_copy(out=pooled[:], in_=red[:])
                else:
                    nc.vector.tensor_max(out=pooled[:], in0=pooled[:],
                                         in1=red[:])
                firstp = False
                pcol += w + 1
            if stage < 4:
                nc.vector.memset(pooled[:], 0.0)
            if stage < 5:
                nc.sync.dma_start(out=out_t[0:min(128, m['n_graphs']), 0:NCL],
                                  in_=pooled[0:min(128, m['n_graphs']), 0:NCL])
            if stage >= 5:
              nc.gpsimd.indirect_dma_start(
                out=pool_scat[:, :],
                out_offset=bass.IndirectOffsetOnAxis(ap=scat_sb[:, 0:1],
                                                     axis=0),
                in_=pooled[:],
                in_offset=None)
              nc.gpsimd.collective_compute(
                "AllReduce", mybir.AluOpType.max,
                replica_groups=[list(range(NC))],
                ins=[pool_scat[0:NG, :]], outs=[pool_red[:, :]])

            # ---------------- MLP + log_softmax (replicated)
            if stage < 5:
                nc.compile_marker = None
            gT = wpool.tile([128, KBM, NGT * 128], F32, tag="gT")
            for i in range(NGT) if stage >= 5 else []:
                gtile = wpool.tile([128, HID], F32, tag="gtile")
                r0, r1 = i * 128, min((i + 1) * 128, NG)
                if r1 - r0 < 128:
                    nc.vector.memset(gtile[:], 0.0)
                nc.sync.dma_start(out=gtile[0:r1 - r0, :],
                                  in_=pool_red[r0:r1, :])
                for k in range(KBM):
                    tps = tp_ps.tile([128, 128], F32, tag="tp")
                    nc.tensor.transpose(out=tps[:],
                                        in_=gtile[:, k * 128:(k + 1) * 128],
                                        identity=ident[:])
                    nc.vector.tensor_copy(out=gT[:, k, i * 128:(i + 1) * 128],
                                          in_=tps[:])
            o1T = wpool.tile([128, NGT * 128], F32, tag="o1T")
            for i in range(NGT) if stage >= 5 else []:
                mm1 = mm_ps.tile([128, HMLP], F32, tag="mm")
                nc.tensor.matmul(out=mm1[:], lhsT=ones[0:1, :],
                                 rhs=fcb1_sb[0:1, :], start=True, stop=False)
                for k in range(KBM):
                    nc.tensor.matmul(out=mm1[:],
                                     lhsT=gT[:, k, i * 128:(i + 1) * 128],
                                     rhs=fcW1_sb[:, k, :],
                                     start=False, stop=(k == KBM - 1))
                o1 = wpool.tile([128, HMLP], F32, tag="o1")
                nc.scalar.activation(out=o1[:], in_=mm1[:],
                                     func=mybir.ActivationFunctionType.Relu)
                tps = tp_ps.tile([128, 128], F32, tag="tp")
                nc.tensor.transpose(out=tps[0:HMLP, :], in_=o1[:],
                                    identity=ident[:])
                nc.vector.tensor_copy(out=o1T[0:HMLP, i * 128:(i + 1) * 128],
                                      in_=tps[0:HMLP, :])
            for i in range(NGT) if stage >= 5 else []:
                mm2 = mm_ps.tile([128, NCL], F32, tag="mm2")
                nc.tensor.matmul(out=mm2[:], lhsT=ones[0:1, :],
                                 rhs=fcb2_sb[0:1, :], start=True, stop=False)
                nc.tensor.matmul(out=mm2[:],
                                 lhsT=o1T[0:HMLP, i * 128:(i + 1) * 128],
                                 rhs=fcW2_sb[0:HMLP, :],
                                 start=False, stop=True)
                # log_softmax rows
                mx = wpool.tile([128, 1], F32, tag="mx")
                nc.vector.tensor_reduce(out=mx[:], in_=mm2[:],
                                        axis=mybir.AxisListType.X,
                                        op=mybir.AluOpType.max)
                tsh = wpool.tile([128, NCL], F32, tag="tsh")
                nc.vector.tensor_scalar(
                    out=tsh[:], in0=mm2[:], scalar1=mx[:, 0:1], scalar2=None,
                    op0=mybir.AluOpType.subtract)
                ex = wpool.tile([128, NCL], F32, tag="ex")
                nc.scalar.activation(out=ex[:], in_=tsh[:],
                                     func=mybir.ActivationFunctionType.Exp)
                sm = wpool.tile([128, 1], F32, tag="sm")
                nc.vector.tensor_reduce(out=sm[:], in_=ex[:],
                                        axis=mybir.AxisListType.X,
                                        op=mybir.AluOpType.add)
                ls = wpool.tile([128, 1], F32, tag="ls")
                nc.scalar.activation(out=ls[:], in_=sm[:],
                                     func=mybir.ActivationFunctionType.Ln)
                oo = opool.tile([128, NCL], F32, tag="oo")
                nc.vector.tensor_scalar(
                    out=oo[:], in0=tsh[:], scalar1=ls[:, 0:1], scalar2=None,
                    op0=mybir.AluOpType.subtract)
                r0, r1 = i * 128, min((i + 1) * 128, NG)
                nc.sync.dma_start(out=out_t[r0:r1, :], in_=oo[0:r1 - r0, :])

    nc.compile()
    return nc


# ---------------------------------------------------------------- entry

def _ensure_ntff_hook():
    """Install the axon NTFF profile hook if the image's antenv lacks it.

    Dev-only (trace=True): lets run_bass_kernel_spmd return exec_time_ns.
    """
    import sys as _sys
    import types as _types
    try:
        from antenv.axon_hooks import get_axon_ntff_profile_hook  # noqa
        return
    except ImportError:
        pass
    try:
        _sys.path.insert(0, "/root/.axon_site")
        from trn_agent_boot.trn_boot import _ntff_profile_via_ctypes
        hook = _ntff_profile_via_ctypes("/opt/axon/libaxon_pjrt.so")
        mod = _types.ModuleType("antenv.axon_hooks")
        mod._hook = hook
        mod.get_axon_ntff_profile_hook = lambda: mod._hook
        mod.set_axon_ntff_profile_hook = lambda h: setattr(mod, "_hook", h)
        _sys.modules["antenv.axon_hooks"] = mod
        # artifact upload needs a bucket; degrade to no-op on failure
        _orig_upload = bass_utils.upload_artifacts

        def _safe_upload(tmpdir):
            try:
                return _orig_upload(tmpdir)
            except Exception:
                return tmpdir
        bass_utils.upload_artifacts = _safe_upload
    except Exception:
        pass


def kernel(x, edge_index, batch, W1, b1, W2, b2, fcW1, fcb1, fcW2, fcb2,
           trace=False):
    if trace:
        _ensure_ntff_hook()
    x = np.asarray(x, np.float32)
    edge_index = np.asarray(edge_index, np.int64)
    batch = np.asarray(batch, np.int64)
    NG = 512
    meta, per_core = prep(x, edge_index, batch, NG)

    nc = build(meta, CIN=x.shape[1], HID=W1.shape[1], HMLP=fcW1.shape[1],
               NCL=fcW2.shape[1])

    shared = dict(
        x=x,
        W1=np.asarray(W1, np.float32), b1=np.asarray(b1, np.float32)[None, :],
        W2=np.asarray(W2, np.float32), b2=np.asarray(b2, np.float32)[None, :],
        fcW1=np.asarray(fcW1, np.float32),
        fcb1=np.asarray(fcb1, np.float32)[None, :],
        fcW2=np.asarray(fcW2, np.float32),
        fcb2=np.asarray(fcb2, np.float32)[None, :],
    )
    in_maps = []
    for c in range(meta["NC"]):
        d = dict(shared)
        pc = per_core[c]
        d["idx1"] = pc["idx1"]
        d["idx2"] = pc["idx2"]
        d["dinv1"] = pc["dinv1"]
        d["dinv_dst"] = pc["dinv_dst"]
        d["idxp"] = pc["idxp"]
        d["scat_g"] = np.tile(pc["scat_g"], (1, 1))
        in_maps.append(d)

    res = bass_utils.run_bass_kernel_spmd(
        nc, in_maps, core_ids=list(range(meta["NC"])), trace=trace)
    out = res.results[0]["out"]
    kernel.last_exec_time_ns = res.exec_time_ns
    return out


kernel.last_exec_time_ns = None



# revision 14
# speedup vs baseline: 1.5350x; 1.3724x over previous
"""GCN (2-layer GCNConv + global max pool + MLP + log_softmax) on 8 trn2 cores.

Strategy (sharding_hint: partition nodes + incident edges, replicate weights):
  - Nodes are partitioned 6250/core (+22 pad nodes/core -> 6272 = 49 tiles of
    128). Within each core, nodes are sorted by degree (desc) so that the
    per-tile padded gather width J_t ~= the true degree.
  - Edges are grouped by dst; each core owns edges into its nodes. For each
    128-node tile the messages are fetched with dma_gather (int16 signed
    indices relative to a mid-table base row cover all 50176/50000 rows),
    giving [128 nodes, J, C] tiles which are reduced on DVE.
  - GCN normalization: agg = D^-1/2 (A+I) D^-1/2 h. Layer-1 folds
    dinv[src] into the DVE accumulate; the produced h1 is pre-scaled by
    dinv (h1' = dinv * relu(...)), so layer-2 accumulation is plain adds.
  - h1' shards are AllGathered into a replicated table; layer-2 gathers
    from it. Max pooling is a dma_gather per graph-partition from the local
    h2 shard + DVE max-reduce, scattered into a [513,256] table (indirect
    scatter handles per-core graph offsets), AllReduce(max), then the small
    MLP + log_softmax run replicated on every core.
"""

import numpy as np

import concourse.bass as bass
import concourse.bacc as bacc
import concourse.tile as tile
import concourse.mybir as mybir
from concourse import bass_utils
from concourse.masks import make_identity
from concourse._compat import cdiv

F32 = mybir.dt.float32
I16 = mybir.dt.int16
I32 = mybir.dt.int32

NEG_BIG = -1.0e38


# ---------------------------------------------------------------- host prep

def _wrap_idx(flat):
    """j-major flat int16 idx list [n] -> wrapped SBUF layout [128, n//16].

    dma_gather consumes idx i from wrapped[i % 16, i // 16]; the 16-row
    pattern is replicated to all 128 partitions.
    """
    n = len(flat)
    assert n % 128 == 0
    w = np.zeros((16, n // 16), np.int16)
    w[np.arange(n) % 16, np.arange(n) // 16] = flat
    return np.tile(w, (8, 1))


def prep(x, edge_index, batch, n_graphs, n_cores=8, j_cap=6, pool_cap=6,
         mid_base=True):
    """All index-space preprocessing. Returns (meta, per-core arrays).

    The x table fed to layer 1 is host-prescaled (xs = dinv * x) with a
    trailing zero row at index N, so pad slots gather exact zeros and the
    on-device accumulate is a plain sum for both layers.
    """
    N = x.shape[0]
    NR = N // n_cores                      # real nodes per core
    LV = int(cdiv(NR, 128)) * 128          # padded nodes per core
    T = LV // 128                          # tiles per core
    NP = LV * n_cores                      # padded total
    BASE1 = (N + 1) // 2 if mid_base else 0   # xs-table base row
    BASE2 = NP // 2 if mid_base else 0     # h1-table base row
    assert max(N + 1 - BASE1, BASE1, NP - BASE2, BASE2, NR + 1) <= 32767

    src = np.concatenate([edge_index[0], np.arange(N, dtype=np.int64)])
    dst = np.concatenate([edge_index[1], np.arange(N, dtype=np.int64)])
    deg = np.bincount(dst, minlength=N).astype(np.int64)
    dinv = np.zeros(N, np.float32)
    nz = deg > 0
    dinv[nz] = 1.0 / np.sqrt(np.maximum(deg[nz], 1).astype(np.float32))

    # group edges by dst
    order = np.argsort(dst, kind="stable")
    src_s = src[order]
    starts = np.searchsorted(dst[order], np.arange(N))
    ends = np.searchsorted(dst[order], np.arange(N) + 1)

    # per-core degree-sorted permutation; perm[c][l] = orig id, -1 = pad
    perm = np.full((n_cores, LV), -1, np.int64)
    for c in range(n_cores):
        lo = NR * c
        perm[c, :NR] = np.argsort(-deg[lo:lo + NR], kind="stable") + lo
    perm_row = np.full(N, -1, np.int64)   # orig id -> permuted global row
    for c in range(n_cores):
        perm_row[perm[c, :NR]] = LV * c + np.arange(NR)

    # J_t per tile (max over cores), split into gather chunks of width <=j_cap
    Jt = np.zeros(T, np.int64)
    for c in range(n_cores):
        for t in range(T):
            ids = perm[c, t * 128:(t + 1) * 128]
            ids = ids[ids >= 0]
            if len(ids):
                Jt[t] = max(Jt[t], deg[ids].max())
    Jt = np.maximum(Jt, 1)
    chunks = [[j_cap] * (int(j) // j_cap) + ([int(j) % j_cap] if j % j_cap else [])
              for j in Jt]
    sumJ = int(Jt.sum())

    # pad rows: any pad node's permuted row (h1' there is forced to 0);
    # use the globally-last pad row.
    pad_row2 = NP - 1 if NP > N else None
    assert pad_row2 is not None, "need at least one pad node for L2 padding"

    # per-core slot tables
    per_core = []
    for c in range(n_cores):
        idx1 = np.zeros((sumJ * 128,), np.int16)
        idx2 = np.zeros((sumJ * 128,), np.int16)
        dinv_dst = np.zeros((128, T), np.float32)
        off = 0
        for t in range(T):
            J = int(Jt[t])
            for p in range(128):
                n = perm[c, t * 128 + p]
                if n >= 0:
                    dinv_dst[p, t] = dinv[n]
                    ss = src_s[starts[n]:ends[n]]
                    nj = len(ss)
                    sl = (off + np.arange(nj)) * 128 + p
                    idx1[sl] = (ss - BASE1).astype(np.int16)
                    idx2[sl] = (perm_row[ss] - BASE2).astype(np.int16)
                else:
                    nj = 0
                # pad slots gather exact-zero rows
                if nj < J:
                    sl = (off + np.arange(nj, J)) * 128 + p
                    idx1[sl] = N - BASE1          # zero row of xs
                    idx2[sl] = pad_row2 - BASE2   # zero row
            off += J
        assert off == sumJ

        # wrapped layout per gather chunk; each chunk gets one trailing
        # all-pad block: the gather's final descriptor flakily skips its
        # data write, so the last 128 slots are sacrificial and never read.
        pad_blk = np.zeros(128, np.int16)
        w1 = []
        w2 = []
        off = 0
        for t in range(T):
            for w in chunks[t]:
                blk = slice(off * 128, (off + w) * 128)
                w1.append(_wrap_idx(np.concatenate([idx1[blk], pad_blk])))
                w2.append(_wrap_idx(np.concatenate([idx2[blk], pad_blk])))
                off += w
        idx1_w = np.concatenate(w1, axis=1)
        idx2_w = np.concatenate(w2, axis=1)
        per_core.append(dict(idx1=idx1_w, idx2=idx2_w, dinv_dst=dinv_dst))

    # pooling: per-core graph ranges + member lists (permuted-local rows)
    glo = np.zeros(n_cores, np.int64)
    Gc = np.zeros(n_cores, np.int64)
    for c in range(n_cores):
        b = batch[NR * c:NR * (c + 1)]
        glo[c] = b.min()
        Gc[c] = b.max() - b.min() + 1
    G_max = int(Gc.max())
    assert G_max <= 128
    # member lists
    members = []   # [core][local graph] -> list of local permuted rows
    for c in range(n_cores):
        b = batch[NR * c:NR * (c + 1)]
        loc = [[] for _ in range(G_max)]
        inv_l = np.empty(NR, np.int64)
        inv_l[perm[c, :NR] - NR * c] = np.arange(NR)
        for i in range(NR):
            loc[int(b[i] - glo[c])].append(int(inv_l[i]))
        members.append(loc)
    Jp = max(max(len(m) for m in loc) for loc in members)
    pool_chunks = [pool_cap] * (Jp // pool_cap) + \
        ([Jp % pool_cap] if Jp % pool_cap else [])
    PAD_POOL = LV  # row LV of h2_local = NEG_BIG
    for c in range(n_cores):
        flat = np.full((len(pool_chunks) and sum(pool_chunks)) * 128, PAD_POOL,
                       np.int16)
        loc = members[c]
        for p in range(128):
            mem = loc[p] if p < G_max else []
            for j, r in enumerate(mem):
                flat[j * 128 + p] = r
        w = []
        off = 0
        pad_blk0 = np.zeros(128, np.int16)
        for wdt in pool_chunks:
            blk = slice(off * 128, (off + wdt) * 128)
            w.append(_wrap_idx(np.concatenate([flat[blk], pad_blk0])))
            off += wdt
        per_core[c]["idxp"] = np.concatenate(w, axis=1)
        scat = np.full(128, n_graphs, np.int64)
        scat[:int(Gc[c])] = glo[c] + np.arange(int(Gc[c]))
        per_core[c]["scat_g"] = scat.astype(np.int32)[:, None]

    meta = dict(N=N, NP=NP, LV=LV, T=T, NC=n_cores, BASE1=BASE1, BASE2=BASE2,
                chunks=chunks, sumJ=sumJ, pool_chunks=pool_chunks,
                n_graphs=n_graphs, dinv=dinv)
    return meta, per_core


# ---------------------------------------------------------------- bass build

def build(meta, CIN, HID, HMLP, NCL, stage=5, n_queues=4):
    """Build the SPMD Bass program. All per-core variation flows via inputs."""
    m = meta
    T, NC = m["T"], m["NC"]
    N, NP, LV = m["N"], m["NP"], m["LV"]
    chunks, pool_chunks = m["chunks"], m["pool_chunks"]
    sumJ = m["sumJ"]
    NG = m["n_graphs"]
    NGT = cdiv(NG, 128)          # pooled tiles (4)
    n_chunk_cols = sum(sum(w + 1 for w in cl) for cl in chunks)
    n_pool_cols = sum(w + 1 for w in pool_chunks)

    nc = bacc.Bacc("TRN2", target_bir_lowering=False, debug=False,
                   num_devices=NC, num_swdge_queues=n_queues)
    qctr = [0]

    def next_q():
        q = qctr[0] % n_queues
        qctr[0] += 1
        return q
    dt = mybir.dt

    # ---- inputs
    x_t = nc.dram_tensor("xs", [N + 1, CIN], F32, kind="ExternalInput")
    idx1_t = nc.dram_tensor("idx1", [128, n_chunk_cols * 8], I16,
                            kind="ExternalInput")
    idx2_t = nc.dram_tensor("idx2", [128, n_chunk_cols * 8], I16,
                            kind="ExternalInput")
    dinvd_t = nc.dram_tensor("dinv_dst", [128, T], F32, kind="ExternalInput")
    idxp_t = nc.dram_tensor("idxp", [128, n_pool_cols * 8], I16,
                            kind="ExternalInput")
    scat_t = nc.dram_tensor("scat_g", [128, 1], I32, kind="ExternalInput")
    W1_t = nc.dram_tensor("W1", [CIN, HID], F32, kind="ExternalInput")
    b1_t = nc.dram_tensor("b1", [1, HID], F32, kind="ExternalInput")
    W2_t = nc.dram_tensor("W2", [HID, HID], F32, kind="ExternalInput")
    b2_t = nc.dram_tensor("b2", [1, HID], F32, kind="ExternalInput")
    fcW1_t = nc.dram_tensor("fcW1", [HID, HMLP], F32, kind="ExternalInput")
    fcb1_t = nc.dram_tensor("fcb1", [1, HMLP], F32, kind="ExternalInput")
    fcW2_t = nc.dram_tensor("fcW2", [HMLP, NCL], F32, kind="ExternalInput")
    fcb2_t = nc.dram_tensor("fcb2", [1, NCL], F32, kind="ExternalInput")
    out_t = nc.dram_tensor("out", [NG, NCL], F32, kind="ExternalOutput")

    KB1 = CIN // 128    # K blocks layer1 (1)
    KB2 = HID // 128    # K blocks layer2 (2)
    KBM = HID // 128    # fc1 K blocks (2)

    with tile.TileContext(nc) as tc:
        with (
            tc.tile_pool(name="const", bufs=1) as cpool,
            tc.tile_pool(name="gath", bufs=6) as gpool,
            tc.tile_pool(name="work", bufs=4) as wpool,
            tc.tile_pool(name="outp", bufs=3) as opool,
            tc.tile_pool(name="tp_ps", bufs=2, space="PSUM") as tp_ps,
            tc.tile_pool(name="mm_ps", bufs=2, space="PSUM") as mm_ps,
            tc.tile_pool(name="dram", bufs=1, space="DRAM") as dr,
        ):
            # ---- constants / weights to SBUF
            ident = cpool.tile([128, 128], F32)
            make_identity(nc, ident[:])
            ones = cpool.tile([1, 128], F32)
            nc.vector.memset(ones[:], 1.0)
            negbig = cpool.tile([128, HID], F32)
            nc.vector.memset(negbig[:], NEG_BIG)

            W1_sb = cpool.tile([128, KB1, HID], F32)
            for k in range(KB1):
                nc.sync.dma_start(out=W1_sb[:, k, :],
                                  in_=W1_t[k * 128:(k + 1) * 128, :])
            W2_sb = cpool.tile([128, KB2, HID], F32)
            for k in range(KB2):
                nc.sync.dma_start(out=W2_sb[:, k, :],
                                  in_=W2_t[k * 128:(k + 1) * 128, :])
            fcW1_sb = cpool.tile([128, KBM, HMLP], F32)
            for k in range(KBM):
                nc.sync.dma_start(out=fcW1_sb[:, k, :],
                                  in_=fcW1_t[k * 128:(k + 1) * 128, :])
            fcW2_sb = cpool.tile([128, NCL], F32)
            nc.sync.dma_start(out=fcW2_sb[:], in_=fcW2_t[:, :])
            b1_sb = cpool.tile([1, HID], F32)
            nc.sync.dma_start(out=b1_sb[:], in_=b1_t[:, :])
            b2_sb = cpool.tile([1, HID], F32)
            nc.sync.dma_start(out=b2_sb[:], in_=b2_t[:, :])
            fcb1_sb = cpool.tile([1, HMLP], F32)
            nc.sync.dma_start(out=fcb1_sb[:], in_=fcb1_t[:, :])
            fcb2_sb = cpool.tile([1, NCL], F32)
            nc.sync.dma_start(out=fcb2_sb[:], in_=fcb2_t[:, :])

            idx1_sb = cpool.tile([128, n_chunk_cols * 8], I16)
            nc.sync.dma_start(out=idx1_sb[:], in_=idx1_t[:, :])
            idx2_sb = cpool.tile([128, n_chunk_cols * 8], I16)
            nc.sync.dma_start(out=idx2_sb[:], in_=idx2_t[:, :])
            dinvd_sb = cpool.tile([128, T], F32)
            nc.sync.dma_start(out=dinvd_sb[:], in_=dinvd_t[:, :])
            idxp_sb = cpool.tile([128, n_pool_cols * 8], I16)
            nc.sync.dma_start(out=idxp_sb[:], in_=idxp_t[:, :])
            scat_sb = cpool.tile([128, 1], I32)
            nc.sync.dma_start(out=scat_sb[:], in_=scat_t[:, :])

            # ---- internal DRAM
            h1_shard = dr.tile([LV, HID], F32)
            h1_table = dr.tile([NP, HID], F32, addr_space="Shared")
            h2_local = dr.tile([LV + 1, HID], F32)
            pool_scat = dr.tile([NG + 1, HID], F32)
            pool_red = dr.tile([NG, HID], F32, addr_space="Shared")

            # init h2 pad row + pool_scat table to NEG_BIG
            nc.sync.dma_start(out=h2_local[LV:LV + 1, :], in_=negbig[0:1, :])
            for i in range(cdiv(NG + 1, 128)):
                r0 = i * 128
                r1 = min(r0 + 128, NG + 1)
                nc.sync.dma_start(out=pool_scat[r0:r1, :],
                                  in_=negbig[0:r1 - r0, :])

            # ---------------- layer helper
            def gcn_layer(layer):
                if layer == 1:
                    C = CIN
                    idx_sb = idx1_sb
                    table_ap = x_t[:, :]
                    KB, W_sb, b_sb = KB1, W1_sb, b1_sb
                    dst_dram = h1_shard
                else:
                    C = HID
                    idx_sb = idx2_sb
                    table_ap = h1_table[:, :]
                    KB, W_sb, b_sb = KB2, W2_sb, b2_sb
                    dst_dram = h2_local
                base = m["BASE1"] if layer == 1 else m["BASE2"]

                icol = 0   # idx column offset (units of 8 int16 per slot)
                gtag = "g1" if layer == 1 else "g2"
                for t in range(T):
                    acc = wpool.tile([128, HID], F32, tag="acc")
                    first = True
                    for w in chunks[t]:
                        g = gpool.tile([128, 8, C], F32, tag=gtag)
                        nc.gpsimd.dma_gather(
                            g[:, 0:w + 1, 0:C],
                            table_ap[base:, :],
                            idx_sb[:, icol * 8:(icol + w + 1) * 8],
                            (w + 1) * 128, (w + 1) * 128, C,
                            queue_num=next_q())
                        # sum over the chunk's slots in one strided reduce
                        if w == 1:
                            if first:
                                nc.vector.tensor_copy(out=acc[:, 0:C],
                                                      in_=g[:, 0, 0:C])
                            else:
                                nc.vector.tensor_add(
                                    out=acc[:, 0:C], in0=acc[:, 0:C],
                                    in1=g[:, 0, 0:C])
                        else:
                            gv = g[:, 0:w, 0:C].rearrange("p j c -> p c j")
                            if first:
                                nc.vector.tensor_reduce(
                                    out=acc[:, 0:C], in_=gv,
                                    axis=mybir.AxisListType.X,
                                    op=mybir.AluOpType.add)
                            else:
                                red = wpool.tile([128, HID], F32, tag="red")
                                nc.vector.tensor_reduce(
                                    out=red[:, 0:C], in_=gv,
                                    axis=mybir.AxisListType.X,
                                    op=mybir.AluOpType.add)
                                nc.vector.tensor_add(
                                    out=acc[:, 0:C], in0=acc[:, 0:C],
                                    in1=red[:, 0:C])
                        first = False
                        icol += w + 1

                    # dst-side dinv scaling
                    nc.vector.tensor_scalar_mul(
                        out=acc[:, 0:C], in0=acc[:, 0:C],
                        scalar1=dinvd_sb[:, t:t + 1])

                    # transpose -> lhsT blocks
                    accT = wpool.tile([128, KB, 128], F32, tag="accT")
                    for k in range(KB):
                        tps = tp_ps.tile([128, 128], F32, tag="tp")
                        nc.tensor.transpose(out=tps[:],
                                            in_=acc[:, k * 128:(k + 1) * 128],
                                            identity=ident[:])
                        nc.vector.tensor_copy(out=accT[:, k, :], in_=tps[:])

                    # matmul: bias + sum_k accT_k.T @ W_k
                    mm = mm_ps.tile([128, HID], F32, tag="mm")
                    nc.tensor.matmul(out=mm[:], lhsT=ones[0:1, :],
                                     rhs=b_sb[0:1, :], start=True, stop=False)
                    for k in range(KB):
                        nc.tensor.matmul(out=mm[:], lhsT=accT[:, k, :],
                                         rhs=W_sb[:, k, :],
                                         start=False, stop=(k == KB - 1))

                    h = opool.tile([128, HID], F32, tag="h")
                    if layer == 1:
                        # h1' = relu(dinv * (aggW + b)) = dinv * relu(aggW+b)
                        nc.scalar.activation(
                            out=h[:], in_=mm[:],
                            func=mybir.ActivationFunctionType.Relu,
                            scale=dinvd_sb[:, t:t + 1])
                    else:
                        nc.scalar.activation(
                            out=h[:], in_=mm[:],
                            func=mybir.ActivationFunctionType.Relu)
                    nc.sync.dma_start(
                        out=dst_dram[t * 128:(t + 1) * 128, :], in_=h[:])

            # ---------------- layer 1 + allgather
            gcn_layer(1)
            if stage >= 2:
              nc.gpsimd.collective_compute(
                "AllGather", mybir.AluOpType.bypass,
                replica_groups=[list(range(NC))],
                ins=[h1_shard[:, :]], outs=[h1_table[:, :]])

            # ---------------- layer 2
            if stage >= 3:
              gcn_layer(2)

            # ---------------- pooling
            pooled = wpool.tile([128, HID], F32, tag="pooled")
            if stage >= 4:
             pcol = 0
             firstp = True
             for w in pool_chunks:
                gp = gpool.tile([128, 8, HID], F32, tag="g2")
                nc.gpsimd.dma_gather(
                    gp[:, 0:w + 1, :],
                    h2_local[0:, :],
                    idxp_sb[:, pcol * 8:(pcol + w + 1) * 8],
                    (w + 1) * 128, (w + 1) * 128, HID,
                    queue_num=next_q())
                # strided view [128, HID, w]: reduce innermost (w)
                red = wpool.tile([128, HID], F32, tag="red")
                gv = gp[:, 0:w, :].rearrange("p j c -> p c j")
                nc.vector.tensor_reduce(out=red[:], in_=gv,
                                        axis=mybir.AxisListType.X,
                                        op=mybir.AluOpType.max)
                if firstp:
                    nc.vector.tensor_copy(out=pooled[:], in_=red[:])
                else:
                    nc.vector.tensor_max(out=pooled[:], in0=pooled[:],
                                         in1=red[:])
                firstp = False
                pcol += w + 1
            if stage < 4:
                nc.vector.memset(pooled[:], 0.0)
            if stage < 5:
                nc.sync.dma_start(out=out_t[0:min(128, m['n_graphs']), 0:NCL],
                                  in_=pooled[0:min(128, m['n_graphs']), 0:NCL])
            if stage >= 5:
              nc.gpsimd.indirect_dma_start(
                out=pool_scat[:, :],
                out_offset=bass.IndirectOffsetOnAxis(ap=scat_sb[:, 0:1],
                                                     axis=0),
                in_=pooled[:],
                in_offset=None)
              nc.gpsimd.collective_compute(
                "AllReduce", mybir.AluOpType.max,
                replica_groups=[list(range(NC))],
                ins=[pool_scat[0:NG, :]], outs=[pool_red[:, :]])

            # ---------------- MLP + log_softmax (replicated)
            if stage < 5:
                nc.compile_marker = None
            gT = wpool.tile([128, KBM, NGT * 128], F32, tag="gT")
            for i in range(NGT) if stage >= 5 else []:
                gtile = wpool.tile([128, HID], F32, tag="gtile")
                r0, r1 = i * 128, min((i + 1) * 128, NG)
                if r1 - r0 < 128:
                    nc.vector.memset(gtile[:], 0.0)
                nc.sync.dma_start(out=gtile[0:r1 - r0, :],
                                  in_=pool_red[r0:r1, :])
                for k in range(KBM):
                    tps = tp_ps.tile([128, 128], F32, tag="tp")
                    nc.tensor.transpose(out=tps[:],
                                        in_=gtile[:, k * 128:(k + 1) * 128],
                                        identity=ident[:])
                    nc.vector.tensor_copy(out=gT[:, k, i * 128:(i + 1) * 128],
                                          in_=tps[:])
            o1T = wpool.tile([128, NGT * 128], F32, tag="o1T")
            for i in range(NGT) if stage >= 5 else []:
                mm1 = mm_ps.tile([128, HMLP], F32, tag="mm")
                nc.tensor.matmul(out=mm1[:], lhsT=ones[0:1, :],
                                 rhs=fcb1_sb[0:1, :], start=True, stop=False)
                for k in range(KBM):
                    nc.tensor.matmul(out=mm1[:],
                                     lhsT=gT[:, k, i * 128:(i + 1) * 128],
                                     rhs=fcW1_sb[:, k, :],
                                     start=False, stop=(k == KBM - 1))
                o1 = wpool.tile([128, HMLP], F32, tag="o1")
                nc.scalar.activation(out=o1[:], in_=mm1[:],
                                     func=mybir.ActivationFunctionType.Relu)
                tps = tp_ps.tile([128, 128], F32, tag="tp")
                nc.tensor.transpose(out=tps[0:HMLP, :], in_=o1[:],
                                    identity=ident[:])
                nc.vector.tensor_copy(out=o1T[0:HMLP, i * 128:(i + 1) * 128],
                                      in_=tps[0:HMLP, :])
            for i in range(NGT) if stage >= 5 else []:
                mm2 = mm_ps.tile([128, NCL], F32, tag="mm2")
                nc.tensor.matmul(out=mm2[:], lhsT=ones[0:1, :],
                                 rhs=fcb2_sb[0:1, :], start=True, stop=False)
                nc.tensor.matmul(out=mm2[:],
                                 lhsT=o1T[0:HMLP, i * 128:(i + 1) * 128],
                                 rhs=fcW2_sb[0:HMLP, :],
                                 start=False, stop=True)
                # log_softmax rows
                mx = wpool.tile([128, 1], F32, tag="mx")
                nc.vector.tensor_reduce(out=mx[:], in_=mm2[:],
                                        axis=mybir.AxisListType.X,
                                        op=mybir.AluOpType.max)
                tsh = wpool.tile([128, NCL], F32, tag="tsh")
                nc.vector.tensor_scalar(
                    out=tsh[:], in0=mm2[:], scalar1=mx[:, 0:1], scalar2=None,
                    op0=mybir.AluOpType.subtract)
                ex = wpool.tile([128, NCL], F32, tag="ex")
                nc.scalar.activation(out=ex[:], in_=tsh[:],
                                     func=mybir.ActivationFunctionType.Exp)
                sm = wpool.tile([128, 1], F32, tag="sm")
                nc.vector.tensor_reduce(out=sm[:], in_=ex[:],
                                        axis=mybir.AxisListType.X,
                                        op=mybir.AluOpType.add)
                ls = wpool.tile([128, 1], F32, tag="ls")
                nc.scalar.activation(out=ls[:], in_=sm[:],
                                     func=mybir.ActivationFunctionType.Ln)
                oo = opool.tile([128, NCL], F32, tag="oo")
                nc.vector.tensor_scalar(
                    out=oo[:], in0=tsh[:], scalar1=ls[:, 0:1], scalar2=None,
                    op0=mybir.AluOpType.subtract)
                r0, r1 = i * 128, min((i + 1) * 128, NG)
                nc.sync.dma_start(out=out_t[r0:r1, :], in_=oo[0:r1 - r0, :])

    nc.compile()
    return nc


# ---------------------------------------------------------------- entry

def _ensure_ntff_hook():
    """Install the axon NTFF profile hook if the image's antenv lacks it.

    Dev-only (trace=True): lets run_bass_kernel_spmd return exec_time_ns.
    """
    import sys as _sys
    import types as _types
    try:
        from antenv.axon_hooks import get_axon_ntff_profile_hook  # noqa
        return
    except ImportError:
        pass
    try:
        _sys.path.insert(0, "/root/.axon_site")
        from trn_agent_boot.trn_boot import _ntff_profile_via_ctypes
        hook = _ntff_profile_via_ctypes("/opt/axon/libaxon_pjrt.so")
        mod = _types.ModuleType("antenv.axon_hooks")
        mod._hook = hook
        mod.get_axon_ntff_profile_hook = lambda: mod._hook
        mod.set_axon_ntff_profile_hook = lambda h: setattr(mod, "_hook", h)
        _sys.modules["antenv.axon_hooks"] = mod
        # artifact upload needs a bucket; degrade to no-op on failure
        _orig_upload = bass_utils.upload_artifacts

        def _safe_upload(tmpdir):
            try:
                return _orig_upload(tmpdir)
            except Exception:
                return tmpdir
        bass_utils.upload_artifacts = _safe_upload
    except Exception:
        pass


def kernel(x, edge_index, batch, W1, b1, W2, b2, fcW1, fcb1, fcW2, fcb2,
           trace=False):
    if trace:
        _ensure_ntff_hook()
    x = np.asarray(x, np.float32)
    edge_index = np.asarray(edge_index, np.int64)
    batch = np.asarray(batch, np.int64)
    NG = 512
    meta, per_core = prep(x, edge_index, batch, NG)

    nc = build(meta, CIN=x.shape[1], HID=W1.shape[1], HMLP=fcW1.shape[1],
               NCL=fcW2.shape[1])

    xs = np.concatenate([meta["dinv"][:, None] * x,
                         np.zeros((1, x.shape[1]), np.float32)], axis=0)
    shared = dict(
        xs=xs,
        W1=np.asarray(W1, np.float32), b1=np.asarray(b1, np.float32)[None, :],
        W2=np.asarray(W2, np.float32), b2=np.asarray(b2, np.float32)[None, :],
        fcW1=np.asarray(fcW1, np.float32),
        fcb1=np.asarray(fcb1, np.float32)[None, :],
        fcW2=np.asarray(fcW2, np.float32),
        fcb2=np.asarray(fcb2, np.float32)[None, :],
    )
    in_maps = []
    for c in range(meta["NC"]):
        d = dict(shared)
        pc = per_core[c]
        d["idx1"] = pc["idx1"]
        d["idx2"] = pc["idx2"]
        d["dinv_dst"] = pc["dinv_dst"]
        d["idxp"] = pc["idxp"]
        d["scat_g"] = np.tile(pc["scat_g"], (1, 1))
        in_maps.append(d)

    res = bass_utils.run_bass_kernel_spmd(
        nc, in_maps, core_ids=list(range(meta["NC"])), trace=trace)
    out = res.results[0]["out"]
    kernel.last_exec_time_ns = res.exec_time_ns
    return out


kernel.last_exec_time_ns = None



# revision 29
# speedup vs baseline: 1.7643x; 1.1494x over previous
"""GCN (2-layer GCNConv + global max pool + MLP + log_softmax) on 8 trn2 cores.

Strategy (sharding_hint: partition nodes + incident edges, replicate weights):
  - Nodes are partitioned 6250/core (+22 pad nodes/core -> 6272 = 49 tiles of
    128). Within each core, nodes are sorted by degree (desc) so that the
    per-tile padded gather width J_t ~= the true degree.
  - Edges are grouped by dst; each core owns edges into its nodes. For each
    128-node tile the messages are fetched with dma_gather (int16 signed
    indices relative to a mid-table base row cover all 50176/50000 rows),
    giving [128 nodes, J, C] tiles which are reduced on DVE.
  - GCN normalization: agg = D^-1/2 (A+I) D^-1/2 h. Layer-1 folds
    dinv[src] into the DVE accumulate; the produced h1 is pre-scaled by
    dinv (h1' = dinv * relu(...)), so layer-2 accumulation is plain adds.
  - h1' shards are AllGathered into a replicated table; layer-2 gathers
    from it. Max pooling is a dma_gather per graph-partition from the local
    h2 shard + DVE max-reduce, scattered into a [513,256] table (indirect
    scatter handles per-core graph offsets), AllReduce(max), then the small
    MLP + log_softmax run replicated on every core.
"""

import numpy as np

import concourse.bass as bass
import concourse.bacc as bacc
import concourse.tile as tile
import concourse.mybir as mybir
from concourse import bass_utils
from concourse.masks import make_identity
from concourse._compat import cdiv

F32 = mybir.dt.float32
BF16 = mybir.dt.bfloat16
I16 = mybir.dt.int16
I32 = mybir.dt.int32

NEG_BIG = -1.0e38


# ---------------------------------------------------------------- host prep

def _wrap_idx(flat):
    """j-major flat int16 idx list [n] -> wrapped SBUF layout [128, n//16].

    dma_gather consumes idx i from wrapped[i % 16, i // 16]; the 16-row
    pattern is replicated to all 128 partitions.
    """
    n = len(flat)
    assert n % 128 == 0
    w = np.zeros((16, n // 16), np.int16)
    w[np.arange(n) % 16, np.arange(n) // 16] = flat
    return np.tile(w, (8, 1))


def prep(x, edge_index, batch, n_graphs, n_cores=8, j_cap=6, n_ag_chunks=4,
         mid_base=True):
    """All index-space preprocessing. Returns (meta, per-core arrays).

    - The x table fed to layer 1 is host-prescaled (xs = dinv * x) with a
      trailing zero row at index N, so pad slots gather exact zeros and the
      on-device accumulate is a plain sum for both layers.
    - The reference's added self-loops are NOT emitted as gather slots; the
      kernel adds the local (permuted-sequential) row per tile instead.
    - The h1 table is laid out AllGather-chunk-major: chunk k holds rows
      [NC * r0_k, NC * r1_k) as [core][local row] so each chunked AllGather
      writes a contiguous range.
    """
    N = x.shape[0]
    NR = N // n_cores                      # real nodes per core
    LV = int(cdiv(NR, 128)) * 128          # padded nodes per core
    T = LV // 128                          # tiles per core
    NP = LV * n_cores                      # padded total
    BASE1 = (N + 1) // 2 if mid_base else 0   # xs-table base row
    BASE2 = NP // 2 if mid_base else 0     # h1-table base row
    assert max(N + 1 - BASE1, BASE1, NP - BASE2, BASE2, NR + 1) <= 32767

    src_e = np.asarray(edge_index[0])
    dst_e = np.asarray(edge_index[1])
    deg = np.bincount(dst_e, minlength=N).astype(np.int64) + 1  # + self-loop
    dinv = (1.0 / np.sqrt(deg.astype(np.float32))).astype(np.float32)

    # group non-self edges by dst
    order = np.argsort(dst_e, kind="stable")
    src_s = src_e[order]
    starts = np.searchsorted(dst_e[order], np.arange(N))
    ends = np.searchsorted(dst_e[order], np.arange(N) + 1)

    # per-core degree-sorted permutation; perm[c][l] = orig id, -1 = pad
    perm = np.full((n_cores, LV), -1, np.int64)
    for c in range(n_cores):
        lo = NR * c
        perm[c, :NR] = np.argsort(-deg[lo:lo + NR], kind="stable") + lo

    # AllGather chunk boundaries (in tiles -> local rows)
    bt = [round(k * T / n_ag_chunks) for k in range(n_ag_chunks + 1)]
    ag_rows = [(bt[k] * 128, bt[k + 1] * 128) for k in range(n_ag_chunks)]
    # orig id -> h1-table row (chunk-major AllGather layout)
    row_of = np.zeros(LV, np.int64)        # local row -> table row offset fn
    for (r0, r1) in ag_rows:
        row_of[r0:r1] = n_cores * r0 + np.arange(r1 - r0)
    chunk_len = np.zeros(LV, np.int64)
    for (r0, r1) in ag_rows:
        chunk_len[r0:r1] = r1 - r0
    perm_row = np.full(N, -1, np.int64)   # orig id -> h1-table row
    for c in range(n_cores):
        loc = np.arange(NR)
        perm_row[perm[c, :NR]] = row_of[loc] + c * chunk_len[loc]

    # J_t per tile (max over cores) of NON-SELF in-degree
    nsd = ends - starts
    Jt = np.zeros(T, np.int64)
    for c in range(n_cores):
        for t in range(T):
            ids = perm[c, t * 128:(t + 1) * 128]
            ids = ids[ids >= 0]
            if len(ids):
                Jt[t] = max(Jt[t], nsd[ids].max())
    Jt = np.maximum(Jt, 1)
    chunks = [[j_cap] * (int(j) // j_cap) + ([int(j) % j_cap] if j % j_cap else [])
              for j in Jt]
    sumJ = int(Jt.sum())

    # pad rows: any pad node's permuted row (h1' there is forced to 0);
    # the globally-last pad row lands at table row NP - 1 in every layout.
    pad_row2 = NP - 1 if NP > N else None
    assert pad_row2 is not None, "need at least one pad node for L2 padding"

    # per-core slot tables
    per_core = []
    for c in range(n_cores):
        idx1 = np.zeros((sumJ * 128,), np.int16)
        idx2 = np.zeros((sumJ * 128,), np.int16)
        dinv_dst = np.zeros((128, T), np.float32)
        off = 0
        for t in range(T):
            J = int(Jt[t])
            for p in range(128):
                n = perm[c, t * 128 + p]
                if n >= 0:
                    dinv_dst[p, t] = dinv[n]
                    ss = src_s[starts[n]:ends[n]]
                    nj = len(ss)
                    sl = (off + np.arange(nj)) * 128 + p
                    idx1[sl] = (ss - BASE1).astype(np.int16)
                    idx2[sl] = (perm_row[ss] - BASE2).astype(np.int16)
                else:
                    nj = 0
                # pad slots gather exact-zero rows
                if nj < J:
                    sl = (off + np.arange(nj, J)) * 128 + p
                    idx1[sl] = N - BASE1          # zero row of xs
                    idx2[sl] = pad_row2 - BASE2   # zero row
            off += J
        assert off == sumJ

        # wrapped layout per gather chunk; each chunk gets one trailing
        # all-pad block: the gather's final descriptor flakily skips its
        # data write, so the last 128 slots are sacrificial and never read.
        pad_blk = np.zeros(128, np.int16)
        w1 = []
        w2 = []
        off = 0
        for t in range(T):
            for w in chunks[t]:
                blk = slice(off * 128, (off + w) * 128)
                w1.append(_wrap_idx(np.concatenate([idx1[blk], pad_blk])))
                w2.append(_wrap_idx(np.concatenate([idx2[blk], pad_blk])))
                off += w
        idx1_w = np.concatenate(w1, axis=1)
        idx2_w = np.concatenate(w2, axis=1)
        per_core.append(dict(idx1=idx1_w, idx2=idx2_w, dinv_dst=dinv_dst))

    # pooling: h2 rows are scattered into a [j-slot, local graph] layout,
    # then max-reduced over j-slots.  GP/Jp are maxed over cores (SPMD).
    glo = np.zeros(n_cores, np.int64)
    Gc = np.zeros(n_cores, np.int64)
    for c in range(n_cores):
        b = batch[NR * c:NR * (c + 1)]
        glo[c] = b.min()
        Gc[c] = b.max() - b.min() + 1
    GP = int(Gc.max())
    assert GP <= 128
    # member slot j for each local permuted row
    jslot = []
    Jp = 0
    for c in range(n_cores):
        b = batch[NR * c:NR * (c + 1)]
        cnt = np.zeros(GP, np.int64)
        js = np.full(LV, -1, np.int64)
        gl = np.full(LV, -1, np.int64)
        for l in range(LV):
            node = perm[c, l]
            if node >= 0:
                g = int(batch[node] - glo[c])
                js[l] = cnt[g]
                gl[l] = g
                cnt[g] += 1
        jslot.append((js, gl))
        Jp = max(Jp, int(cnt.max()))
    for c in range(n_cores):
        js, gl = jslot[c]
        scat_rows = np.full((128, T), Jp * GP, np.int32)  # dump row
        for l in range(LV):
            if js[l] >= 0:
                scat_rows[l % 128, l // 128] = js[l] * GP + gl[l]
        per_core[c]["scat_rows"] = scat_rows
        scat = np.full(128, n_graphs, np.int64)
        scat[:int(Gc[c])] = glo[c] + np.arange(int(Gc[c]))
        per_core[c]["scat_g"] = scat.astype(np.int32)[:, None]

    meta = dict(N=N, NP=NP, LV=LV, T=T, NC=n_cores, BASE1=BASE1, BASE2=BASE2,
                chunks=chunks, sumJ=sumJ, n_graphs=n_graphs, dinv=dinv,
                GP=GP, Jp=Jp, ag_rows=ag_rows, perm=perm)
    return meta, per_core


# ---------------------------------------------------------------- bass build

def build(meta, CIN, HID, HMLP, NCL, n_queues=4):
    """Build the SPMD Bass program. All per-core variation flows via inputs."""
    m = meta
    T, NC = m["T"], m["NC"]
    N, NP, LV = m["N"], m["NP"], m["LV"]
    chunks = m["chunks"]
    GP, Jp, ag_rows = m["GP"], m["Jp"], m["ag_rows"]
    NG = m["n_graphs"]
    NGT = cdiv(NG, 128)          # pooled tiles (4)
    n_chunk_cols = sum(sum(w + 1 for w in cl) for cl in chunks)

    nc = bacc.Bacc("TRN2", target_bir_lowering=False, debug=False,
                   num_devices=NC, num_swdge_queues=n_queues)
    qctr = [0]

    def next_q():
        q = qctr[0] % n_queues
        qctr[0] += 1
        return q
    dt = mybir.dt

    # ---- inputs
    x_t = nc.dram_tensor("xs", [N + 1, CIN], BF16, kind="ExternalInput")
    xsp_t = nc.dram_tensor("xsp", [LV, CIN], BF16, kind="ExternalInput")
    idx1_t = nc.dram_tensor("idx1", [128, n_chunk_cols * 8], I16,
                            kind="ExternalInput")
    idx2_t = nc.dram_tensor("idx2", [128, n_chunk_cols * 8], I16,
                            kind="ExternalInput")
    dinvd_t = nc.dram_tensor("dinv_dst", [128, T], F32, kind="ExternalInput")
    scatr_t = nc.dram_tensor("scat_rows", [128, T], I32, kind="ExternalInput")
    scat_t = nc.dram_tensor("scat_g", [128, 1], I32, kind="ExternalInput")
    W1_t = nc.dram_tensor("W1", [CIN, HID], F32, kind="ExternalInput")
    b1_t = nc.dram_tensor("b1", [1, HID], F32, kind="ExternalInput")
    W2_t = nc.dram_tensor("W2", [HID, HID], F32, kind="ExternalInput")
    b2_t = nc.dram_tensor("b2", [1, HID], F32, kind="ExternalInput")
    fcW1_t = nc.dram_tensor("fcW1", [HID, HMLP], F32, kind="ExternalInput")
    fcb1_t = nc.dram_tensor("fcb1", [1, HMLP], F32, kind="ExternalInput")
    fcW2_t = nc.dram_tensor("fcW2", [HMLP, NCL], F32, kind="ExternalInput")
    fcb2_t = nc.dram_tensor("fcb2", [1, NCL], F32, kind="ExternalInput")
    out_t = nc.dram_tensor("out", [NG, NCL], F32, kind="ExternalOutput")

    KB1 = CIN // 128    # K blocks layer1 (1)
    KB2 = HID // 128    # K blocks layer2 (2)
    KBM = HID // 128    # fc1 K blocks (2)

    with tile.TileContext(nc) as tc:
        with (
            tc.tile_pool(name="const", bufs=1) as cpool,
            tc.tile_pool(name="gath", bufs=6) as gpool,
            tc.tile_pool(name="work", bufs=4) as wpool,
            tc.tile_pool(name="outp", bufs=3) as opool,
            tc.tile_pool(name="tp_ps", bufs=2, space="PSUM") as tp_ps,
            tc.tile_pool(name="mm_ps", bufs=2, space="PSUM") as mm_ps,
            tc.tile_pool(name="dram", bufs=1, space="DRAM") as dr,
        ):
            # ---- constants / weights to SBUF
            ident = cpool.tile([128, 128], F32)
            make_identity(nc, ident[:])
            ones = cpool.tile([1, 128], F32)
            nc.vector.memset(ones[:], 1.0)
            negbig = cpool.tile([128, HID], F32)
            nc.vector.memset(negbig[:], NEG_BIG)

            W1_sb = cpool.tile([128, KB1, HID], F32)
            for k in range(KB1):
                nc.sync.dma_start(out=W1_sb[:, k, :],
                                  in_=W1_t[k * 128:(k + 1) * 128, :])
            W2_sb = cpool.tile([128, KB2, HID], F32)
            for k in range(KB2):
                nc.sync.dma_start(out=W2_sb[:, k, :],
                                  in_=W2_t[k * 128:(k + 1) * 128, :])
            fcW1_sb = cpool.tile([128, KBM, HMLP], F32)
            for k in range(KBM):
                nc.sync.dma_start(out=fcW1_sb[:, k, :],
                                  in_=fcW1_t[k * 128:(k + 1) * 128, :])
            fcW2_sb = cpool.tile([128, NCL], F32)
            nc.sync.dma_start(out=fcW2_sb[:], in_=fcW2_t[:, :])
            b1_sb = cpool.tile([1, HID], F32)
            nc.sync.dma_start(out=b1_sb[:], in_=b1_t[:, :])
            b2_sb = cpool.tile([1, HID], F32)
            nc.sync.dma_start(out=b2_sb[:], in_=b2_t[:, :])
            fcb1_sb = cpool.tile([1, HMLP], F32)
            nc.sync.dma_start(out=fcb1_sb[:], in_=fcb1_t[:, :])
            fcb2_sb = cpool.tile([1, NCL], F32)
            nc.sync.dma_start(out=fcb2_sb[:], in_=fcb2_t[:, :])

            idx1_sb = cpool.tile([128, n_chunk_cols * 8], I16)
            nc.sync.dma_start(out=idx1_sb[:], in_=idx1_t[:, :])
            idx2_sb = cpool.tile([128, n_chunk_cols * 8], I16)
            nc.sync.dma_start(out=idx2_sb[:], in_=idx2_t[:, :])
            dinvd_sb = cpool.tile([128, T], F32)
            nc.sync.dma_start(out=dinvd_sb[:], in_=dinvd_t[:, :])
            scatr_sb = cpool.tile([128, T], I32)
            nc.sync.dma_start(out=scatr_sb[:], in_=scatr_t[:, :])
            scat_sb = cpool.tile([128, 1], I32)
            nc.sync.dma_start(out=scat_sb[:], in_=scat_t[:, :])

            # ---- internal DRAM
            h1_shard = dr.tile([LV, HID], BF16)
            h1_table = dr.tile([NP, HID], BF16)
            ag_out = []
            for agk, (r0, r1) in enumerate(ag_rows):
                agt = dr.tile([NC * (r1 - r0), HID], BF16,
                              addr_space="Shared", name=f"ag_out{agk}")
                ag_out.append(agt)
            JPG = (Jp + 1) * GP            # pool layout rows (+dump space)
            pool_layout = dr.tile([JPG, HID], F32)
            pool_scat = dr.tile([NG + 1, HID], F32)
            pool_red = dr.tile([NG, HID], F32, addr_space="Shared")

            # init pool layout + pool_scat table to NEG_BIG
            for i in range(cdiv(JPG, 128)):
                r0 = i * 128
                r1 = min(r0 + 128, JPG)
                nc.sync.dma_start(out=pool_layout[r0:r1, :],
                                  in_=negbig[0:r1 - r0, :])
            for i in range(cdiv(NG + 1, 128)):
                r0 = i * 128
                r1 = min(r0 + 128, NG + 1)
                nc.sync.dma_start(out=pool_scat[r0:r1, :],
                                  in_=negbig[0:r1 - r0, :])

            # chunked AllGather: chunk k fires once its h1 tiles are written
            ag_done = [False] * len(ag_rows)

            def fire_ag(k):
                r0, r1 = ag_rows[k]
                nc.gpsimd.collective_compute(
                    "AllGather", mybir.AluOpType.bypass,
                    replica_groups=[list(range(NC))],
                    ins=[h1_shard[r0:r1, :]],
                    outs=[ag_out[k][:, :]])
                nc.sync.dma_start(out=h1_table[NC * r0:NC * r1, :],
                                  in_=ag_out[k][:, :])
                ag_done[k] = True

            # ---------------- layer helper
            def gcn_layer(layer):
                if layer == 1:
                    C = CIN
                    idx_sb = idx1_sb
                    table_ap = x_t[:, :]
                    KB, W_sb, b_sb = KB1, W1_sb, b1_sb
                else:
                    C = HID
                    idx_sb = idx2_sb
                    table_ap = h1_table[:, :]
                    KB, W_sb, b_sb = KB2, W2_sb, b2_sb
                base = m["BASE1"] if layer == 1 else m["BASE2"]

                icol = 0   # idx column offset (units of 8 int16 per slot)
                gtag = "g1" if layer == 1 else "g2"
                for t in range(T):
                    acc = wpool.tile([128, HID], F32, tag="acc")
                    first = True
                    for w in chunks[t]:
                        g = gpool.tile([128, 8, C], BF16, tag=gtag)
                        nc.gpsimd.dma_gather(
                            g[:, 0:w + 1, 0:C],
                            table_ap[base:, :],
                            idx_sb[:, icol * 8:(icol + w + 1) * 8],
                            (w + 1) * 128, (w + 1) * 128, C,
                            queue_num=next_q())
                        # sum over the chunk's slots in one strided reduce
                        gv = g[:, 0:w, 0:C].rearrange("p j c -> p c j")
                        if first:
                            nc.vector.tensor_reduce(
                                out=acc[:, 0:C], in_=gv,
                                axis=mybir.AxisListType.X,
                                op=mybir.AluOpType.add)
                        else:
                            red = wpool.tile([128, HID], F32, tag="red")
                            nc.vector.tensor_reduce(
                                out=red[:, 0:C], in_=gv,
                                axis=mybir.AxisListType.X,
                                op=mybir.AluOpType.add)
                            nc.vector.tensor_add(
                                out=acc[:, 0:C], in0=acc[:, 0:C],
                                in1=red[:, 0:C])
                        first = False
                        icol += w + 1

                    # self-loop term: local (permuted-sequential) rows
                    sl = wpool.tile([128, HID], BF16, tag="self")
                    if layer == 1:
                        nc.sync.dma_start(
                            out=sl[:, 0:C],
                            in_=xsp_t[t * 128:(t + 1) * 128, :])
                    else:
                        nc.sync.dma_start(
                            out=sl[:, 0:C],
                            in_=h1_shard[t * 128:(t + 1) * 128, :])
                    nc.vector.tensor_add(out=acc[:, 0:C], in0=acc[:, 0:C],
                                         in1=sl[:, 0:C])

                    # dst-side dinv scaling
                    nc.vector.tensor_scalar_mul(
                        out=acc[:, 0:C], in0=acc[:, 0:C],
                        scalar1=dinvd_sb[:, t:t + 1])

                    # transpose -> lhsT blocks
                    accT = wpool.tile([128, KB, 128], F32, tag="accT")
                    for k in range(KB):
                        tps = tp_ps.tile([128, 128], F32, tag="tp")
                        nc.tensor.transpose(out=tps[:],
                                            in_=acc[:, k * 128:(k + 1) * 128],
                                            identity=ident[:])
                        nc.vector.tensor_copy(out=accT[:, k, :], in_=tps[:])

                    # matmul: bias + sum_k accT_k.T @ W_k
                    mm = mm_ps.tile([128, HID], F32, tag="mm")
                    nc.tensor.matmul(out=mm[:], lhsT=ones[0:1, :],
                                     rhs=b_sb[0:1, :], start=True, stop=False)
                    for k in range(KB):
                        nc.tensor.matmul(out=mm[:], lhsT=accT[:, k, :],
                                         rhs=W_sb[:, k, :],
                                         start=False, stop=(k == KB - 1))

                    if layer == 1:
                        # h1' = relu(dinv * (aggW + b)) = dinv * relu(aggW+b)
                        h = opool.tile([128, HID], BF16, tag="h")
                        nc.scalar.activation(
                            out=h[:], in_=mm[:],
                            func=mybir.ActivationFunctionType.Relu,
                            scale=dinvd_sb[:, t:t + 1])
                        nc.sync.dma_start(
                            out=h1_shard[t * 128:(t + 1) * 128, :], in_=h[:])
                        # fire any AllGather chunk whose rows are written
                        # (pipeline lag: wait 3 tiles past the boundary)
                        for k, (r0, r1) in enumerate(ag_rows):
                            if not ag_done[k] and (t + 1) * 128 >= r1 + 640:
                                fire_ag(k)
                    else:
                        h = opool.tile([128, HID], F32, tag="h")
                        nc.scalar.activation(
                            out=h[:], in_=mm[:],
                            func=mybir.ActivationFunctionType.Relu)
                        # scatter rows into the pooling [j-slot, graph] layout
                        nc.gpsimd.indirect_dma_start(
                            out=pool_layout[:, :],
                            out_offset=bass.IndirectOffsetOnAxis(
                                ap=scatr_sb[:, t:t + 1], axis=0),
                            in_=h[:], in_offset=None)

            # ---------------- layer 1 + allgather
            gcn_layer(1)
            for k in range(len(ag_rows)):
                if not ag_done[k]:
                    fire_ag(k)

            # ---------------- layer 2
            gcn_layer(2)

            # ---------------- pooling: max over j-slots of the scatter layout
            pooled = wpool.tile([128, HID], F32, tag="pooled")
            nc.vector.memset(pooled[:], NEG_BIG)
            JC = 8
            for j0 in range(0, Jp, JC):
                jc = min(JC, Jp - j0)
                pt = gpool.tile([128, JC, HID], F32, tag="pool")
                dv = pool_layout[j0 * GP:(j0 + jc) * GP, :].rearrange(
                    "(j g) c -> g j c", j=jc)
                nc.sync.dma_start(out=pt[0:GP, 0:jc, :], in_=dv)
                red = wpool.tile([128, HID], F32, tag="red")
                pv = pt[0:GP, 0:jc, :].rearrange("g j c -> g c j")
                nc.vector.tensor_reduce(out=red[0:GP, :], in_=pv,
                                        axis=mybir.AxisListType.X,
                                        op=mybir.AluOpType.max)
                nc.vector.tensor_max(out=pooled[0:GP, :], in0=pooled[0:GP, :],
                                     in1=red[0:GP, :])
            nc.gpsimd.indirect_dma_start(
                out=pool_scat[:, :],
                out_offset=bass.IndirectOffsetOnAxis(ap=scat_sb[:, 0:1],
                                                     axis=0),
                in_=pooled[:],
                in_offset=None)
            nc.gpsimd.collective_compute(
                "AllReduce", mybir.AluOpType.max,
                replica_groups=[list(range(NC))],
                ins=[pool_scat[0:NG, :]], outs=[pool_red[:, :]])

            # ---------------- MLP + log_softmax (replicated)
            gT = wpool.tile([128, KBM, NGT * 128], F32, tag="gT")
            for i in range(NGT):
                gtile = wpool.tile([128, HID], F32, tag="gtile")
                r0, r1 = i * 128, min((i + 1) * 128, NG)
                if r1 - r0 < 128:
                    nc.vector.memset(gtile[:], 0.0)
                nc.sync.dma_start(out=gtile[0:r1 - r0, :],
                                  in_=pool_red[r0:r1, :])
                for k in range(KBM):
                    tps = tp_ps.tile([128, 128], F32, tag="tp")
                    nc.tensor.transpose(out=tps[:],
                                        in_=gtile[:, k * 128:(k + 1) * 128],
                                        identity=ident[:])
                    nc.vector.tensor_copy(out=gT[:, k, i * 128:(i + 1) * 128],
                                          in_=tps[:])
            o1T = wpool.tile([128, NGT * 128], F32, tag="o1T")
            for i in range(NGT):
                mm1 = mm_ps.tile([128, HMLP], F32, tag="mm")
                nc.tensor.matmul(out=mm1[:], lhsT=ones[0:1, :],
                                 rhs=fcb1_sb[0:1, :], start=True, stop=False)
                for k in range(KBM):
                    nc.tensor.matmul(out=mm1[:],
                                     lhsT=gT[:, k, i * 128:(i + 1) * 128],
                                     rhs=fcW1_sb[:, k, :],
                                     start=False, stop=(k == KBM - 1))
                o1 = wpool.tile([128, HMLP], F32, tag="o1")
                nc.scalar.activation(out=o1[:], in_=mm1[:],
                                     func=mybir.ActivationFunctionType.Relu)
                tps = tp_ps.tile([128, 128], F32, tag="tp")
                nc.tensor.transpose(out=tps[0:HMLP, :], in_=o1[:],
                                    identity=ident[:])
                nc.vector.tensor_copy(out=o1T[0:HMLP, i * 128:(i + 1) * 128],
                                      in_=tps[0:HMLP, :])
            for i in range(NGT):
                mm2 = mm_ps.tile([128, NCL], F32, tag="mm2")
                nc.tensor.matmul(out=mm2[:], lhsT=ones[0:1, :],
                                 rhs=fcb2_sb[0:1, :], start=True, stop=False)
                nc.tensor.matmul(out=mm2[:],
                                 lhsT=o1T[0:HMLP, i * 128:(i + 1) * 128],
                                 rhs=fcW2_sb[0:HMLP, :],
                                 start=False, stop=True)
                # log_softmax rows
                mx = wpool.tile([128, 1], F32, tag="mx")
                nc.vector.tensor_reduce(out=mx[:], in_=mm2[:],
                                        axis=mybir.AxisListType.X,
                                        op=mybir.AluOpType.max)
                tsh = wpool.tile([128, NCL], F32, tag="tsh")
                nc.vector.tensor_scalar(
                    out=tsh[:], in0=mm2[:], scalar1=mx[:, 0:1], scalar2=None,
                    op0=mybir.AluOpType.subtract)
                ex = wpool.tile([128, NCL], F32, tag="ex")
                nc.scalar.activation(out=ex[:], in_=tsh[:],
                                     func=mybir.ActivationFunctionType.Exp)
                sm = wpool.tile([128, 1], F32, tag="sm")
                nc.vector.tensor_reduce(out=sm[:], in_=ex[:],
                                        axis=mybir.AxisListType.X,
                                        op=mybir.AluOpType.add)
                ls = wpool.tile([128, 1], F32, tag="ls")
                nc.scalar.activation(out=ls[:], in_=sm[:],
                                     func=mybir.ActivationFunctionType.Ln)
                oo = opool.tile([128, NCL], F32, tag="oo")
                nc.vector.tensor_scalar(
                    out=oo[:], in0=tsh[:], scalar1=ls[:, 0:1], scalar2=None,
                    op0=mybir.AluOpType.subtract)
                r0, r1 = i * 128, min((i + 1) * 128, NG)
                nc.sync.dma_start(out=out_t[r0:r1, :], in_=oo[0:r1 - r0, :])

    nc.compile()
    return nc


# ---------------------------------------------------------------- entry

def _ensure_ntff_hook():
    """Install the axon NTFF profile hook if the image's antenv lacks it.

    Dev-only (trace=True): lets run_bass_kernel_spmd return exec_time_ns.
    """
    import sys as _sys
    import types as _types
    try:
        from antenv.axon_hooks import get_axon_ntff_profile_hook  # noqa
        return
    except ImportError:
        pass
    try:
        _sys.path.insert(0, "/root/.axon_site")
        from trn_agent_boot.trn_boot import _ntff_profile_via_ctypes
        hook = _ntff_profile_via_ctypes("/opt/axon/libaxon_pjrt.so")
        mod = _types.ModuleType("antenv.axon_hooks")
        mod._hook = hook
        mod.get_axon_ntff_profile_hook = lambda: mod._hook
        mod.set_axon_ntff_profile_hook = lambda h: setattr(mod, "_hook", h)
        _sys.modules["antenv.axon_hooks"] = mod
        # artifact upload needs a bucket; degrade to no-op on failure
        _orig_upload = bass_utils.upload_artifacts

        def _safe_upload(tmpdir):
            try:
                return _orig_upload(tmpdir)
            except Exception:
                return tmpdir
        bass_utils.upload_artifacts = _safe_upload
    except Exception:
        pass


def kernel(x, edge_index, batch, W1, b1, W2, b2, fcW1, fcb1, fcW2, fcb2,
           trace=False):
    if trace:
        _ensure_ntff_hook()
    x = np.asarray(x, np.float32)
    edge_index = np.asarray(edge_index, np.int64)
    batch = np.asarray(batch, np.int64)
    NG = 512
    meta, per_core = prep(x, edge_index, batch, NG)

    nc = build(meta, CIN=x.shape[1], HID=W1.shape[1], HMLP=fcW1.shape[1],
               NCL=fcW2.shape[1])

    import ml_dtypes
    xs = np.concatenate([meta["dinv"][:, None] * x,
                         np.zeros((1, x.shape[1]), np.float32)], axis=0)
    xs_bf = xs.astype(ml_dtypes.bfloat16)
    shared = dict(
        xs=xs_bf,
        W1=np.asarray(W1, np.float32), b1=np.asarray(b1, np.float32)[None, :],
        W2=np.asarray(W2, np.float32), b2=np.asarray(b2, np.float32)[None, :],
        fcW1=np.asarray(fcW1, np.float32),
        fcb1=np.asarray(fcb1, np.float32)[None, :],
        fcW2=np.asarray(fcW2, np.float32),
        fcb2=np.asarray(fcb2, np.float32)[None, :],
    )
    perm = meta["perm"]
    in_maps = []
    for c in range(meta["NC"]):
        d = dict(shared)
        pc = per_core[c]
        # permuted local xs rows (pads -> zero row N)
        pidx = np.where(perm[c] >= 0, perm[c], x.shape[0])
        d["xsp"] = xs_bf[pidx]
        d["idx1"] = pc["idx1"]
        d["idx2"] = pc["idx2"]
        d["dinv_dst"] = pc["dinv_dst"]
        d["scat_rows"] = pc["scat_rows"]
        d["scat_g"] = np.tile(pc["scat_g"], (1, 1))
        in_maps.append(d)

    res = bass_utils.run_bass_kernel_spmd(
        nc, in_maps, core_ids=list(range(meta["NC"])), trace=trace)
    out = res.results[0]["out"]
    kernel.last_exec_time_ns = res.exec_time_ns
    return out


kernel.last_exec_time_ns = None



# revision 30
# speedup vs baseline: 2.1316x; 1.2082x over previous
"""GCN (2-layer GCNConv + global max pool + MLP + log_softmax) on 8 trn2 cores.

Strategy (sharding_hint: partition nodes + incident edges, replicate weights):
  - Nodes are partitioned 6250/core (+22 pad nodes/core -> 6272 = 49 tiles of
    128). Within each core, nodes are sorted by degree (desc) so that the
    per-tile padded gather width J_t ~= the true degree.
  - Edges are grouped by dst; each core owns edges into its nodes. For each
    128-node tile the messages are fetched with dma_gather (int16 signed
    indices relative to a mid-table base row cover all 50176/50000 rows),
    giving [128 nodes, J, C] tiles which are reduced on DVE.
  - GCN normalization: agg = D^-1/2 (A+I) D^-1/2 h. Layer-1 folds
    dinv[src] into the DVE accumulate; the produced h1 is pre-scaled by
    dinv (h1' = dinv * relu(...)), so layer-2 accumulation is plain adds.
  - h1' shards are AllGathered into a replicated table; layer-2 gathers
    from it. Max pooling is a dma_gather per graph-partition from the local
    h2 shard + DVE max-reduce, scattered into a [513,256] table (indirect
    scatter handles per-core graph offsets), AllReduce(max), then the small
    MLP + log_softmax run replicated on every core.
"""

import numpy as np

import concourse.bass as bass
import concourse.bacc as bacc
import concourse.tile as tile
import concourse.mybir as mybir
from concourse import bass_utils
from concourse.masks import make_identity
from concourse._compat import cdiv

F32 = mybir.dt.float32
BF16 = mybir.dt.bfloat16
I16 = mybir.dt.int16
I32 = mybir.dt.int32

NEG_BIG = -1.0e38


# ---------------------------------------------------------------- host prep

def _wrap_idx(flat):
    """j-major flat int16 idx list [n] -> wrapped SBUF layout [128, n//16].

    dma_gather consumes idx i from wrapped[i % 16, i // 16]; the 16-row
    pattern is replicated to all 128 partitions.
    """
    n = len(flat)
    assert n % 128 == 0
    w = np.zeros((16, n // 16), np.int16)
    w[np.arange(n) % 16, np.arange(n) // 16] = flat
    return np.tile(w, (8, 1))


def prep(x, edge_index, batch, n_graphs, n_cores=8, j_cap=6, n_ag_chunks=4,
         mid_base=True):
    """All index-space preprocessing. Returns (meta, per-core arrays).

    - The x table fed to layer 1 is host-prescaled (xs = dinv * x) with a
      trailing zero row at index N, so pad slots gather exact zeros and the
      on-device accumulate is a plain sum for both layers.
    - The reference's added self-loops are NOT emitted as gather slots; the
      kernel adds the local (permuted-sequential) row per tile instead.
    - The h1 table is laid out AllGather-chunk-major: chunk k holds rows
      [NC * r0_k, NC * r1_k) as [core][local row] so each chunked AllGather
      writes a contiguous range.
    """
    N = x.shape[0]
    NR = N // n_cores                      # real nodes per core
    LV = int(cdiv(NR, 128)) * 128          # padded nodes per core
    T = LV // 128                          # tiles per core
    NP = LV * n_cores                      # padded total
    BASE1 = (N + 1) // 2 if mid_base else 0   # xs-table base row
    BASE2 = NP // 2 if mid_base else 0     # h1-table base row
    assert max(N + 1 - BASE1, BASE1, NP - BASE2, BASE2, NR + 1) <= 32767

    src_e = np.asarray(edge_index[0])
    dst_e = np.asarray(edge_index[1])
    deg = np.bincount(dst_e, minlength=N).astype(np.int64) + 1  # + self-loop
    dinv = (1.0 / np.sqrt(deg.astype(np.float32))).astype(np.float32)

    # group non-self edges by dst
    order = np.argsort(dst_e, kind="stable")
    src_s = src_e[order]
    starts = np.searchsorted(dst_e[order], np.arange(N))
    ends = np.searchsorted(dst_e[order], np.arange(N) + 1)

    # per-core degree-sorted permutation; perm[c][l] = orig id, -1 = pad
    perm = np.full((n_cores, LV), -1, np.int64)
    for c in range(n_cores):
        lo = NR * c
        perm[c, :NR] = np.argsort(-deg[lo:lo + NR], kind="stable") + lo

    # AllGather chunk boundaries (in tiles -> local rows)
    bt = [round(k * T / n_ag_chunks) for k in range(n_ag_chunks + 1)]
    ag_rows = [(bt[k] * 128, bt[k + 1] * 128) for k in range(n_ag_chunks)]
    # orig id -> h1-table row (chunk-major AllGather layout)
    row_of = np.zeros(LV, np.int64)        # local row -> table row offset fn
    for (r0, r1) in ag_rows:
        row_of[r0:r1] = n_cores * r0 + np.arange(r1 - r0)
    chunk_len = np.zeros(LV, np.int64)
    for (r0, r1) in ag_rows:
        chunk_len[r0:r1] = r1 - r0
    perm_row = np.full(N, -1, np.int64)   # orig id -> h1-table row
    for c in range(n_cores):
        loc = np.arange(NR)
        perm_row[perm[c, :NR]] = row_of[loc] + c * chunk_len[loc]

    # J_t per tile (max over cores) of NON-SELF in-degree
    nsd = ends - starts
    Jt = np.zeros(T, np.int64)
    for c in range(n_cores):
        for t in range(T):
            ids = perm[c, t * 128:(t + 1) * 128]
            ids = ids[ids >= 0]
            if len(ids):
                Jt[t] = max(Jt[t], nsd[ids].max())
    Jt = np.maximum(Jt, 1)
    chunks = [[j_cap] * (int(j) // j_cap) + ([int(j) % j_cap] if j % j_cap else [])
              for j in Jt]
    sumJ = int(Jt.sum())

    # pad rows: any pad node's permuted row (h1' there is forced to 0);
    # the globally-last pad row lands at table row NP - 1 in every layout.
    pad_row2 = NP - 1 if NP > N else None
    assert pad_row2 is not None, "need at least one pad node for L2 padding"

    # per-core slot tables
    per_core = []
    for c in range(n_cores):
        idx1 = np.zeros((sumJ * 128,), np.int16)
        idx2 = np.zeros((sumJ * 128,), np.int16)
        dinv_dst = np.zeros((128, T), np.float32)
        off = 0
        for t in range(T):
            J = int(Jt[t])
            for p in range(128):
                n = perm[c, t * 128 + p]
                if n >= 0:
                    dinv_dst[p, t] = dinv[n]
                    ss = src_s[starts[n]:ends[n]]
                    nj = len(ss)
                    sl = (off + np.arange(nj)) * 128 + p
                    idx1[sl] = (ss - BASE1).astype(np.int16)
                    idx2[sl] = (perm_row[ss] - BASE2).astype(np.int16)
                else:
                    nj = 0
                # pad slots gather exact-zero rows
                if nj < J:
                    sl = (off + np.arange(nj, J)) * 128 + p
                    idx1[sl] = N - BASE1          # zero row of xs
                    idx2[sl] = pad_row2 - BASE2   # zero row
            off += J
        assert off == sumJ

        # wrapped layout per gather chunk; each chunk gets one trailing
        # all-pad block: the gather's final descriptor flakily skips its
        # data write, so the last 128 slots are sacrificial and never read.
        pad_blk = np.zeros(128, np.int16)
        w1 = []
        w2 = []
        off = 0
        for t in range(T):
            for w in chunks[t]:
                blk = slice(off * 128, (off + w) * 128)
                w1.append(_wrap_idx(np.concatenate([idx1[blk], pad_blk])))
                w2.append(_wrap_idx(np.concatenate([idx2[blk], pad_blk])))
                off += w
        idx1_w = np.concatenate(w1, axis=1)
        idx2_w = np.concatenate(w2, axis=1)
        per_core.append(dict(idx1=idx1_w, idx2=idx2_w, dinv_dst=dinv_dst))

    # pooling: h2 rows are scattered into a [j-slot, local graph] layout,
    # then max-reduced over j-slots.  GP/Jp are maxed over cores (SPMD).
    glo = np.zeros(n_cores, np.int64)
    Gc = np.zeros(n_cores, np.int64)
    for c in range(n_cores):
        b = batch[NR * c:NR * (c + 1)]
        glo[c] = b.min()
        Gc[c] = b.max() - b.min() + 1
    GP = int(Gc.max())
    assert GP <= 128
    # member slot j for each local permuted row
    jslot = []
    Jp = 0
    for c in range(n_cores):
        b = batch[NR * c:NR * (c + 1)]
        cnt = np.zeros(GP, np.int64)
        js = np.full(LV, -1, np.int64)
        gl = np.full(LV, -1, np.int64)
        for l in range(LV):
            node = perm[c, l]
            if node >= 0:
                g = int(batch[node] - glo[c])
                js[l] = cnt[g]
                gl[l] = g
                cnt[g] += 1
        jslot.append((js, gl))
        Jp = max(Jp, int(cnt.max()))
    for c in range(n_cores):
        js, gl = jslot[c]
        scat_rows = np.full((128, T), Jp * GP, np.int32)  # dump row
        for l in range(LV):
            if js[l] >= 0:
                scat_rows[l % 128, l // 128] = js[l] * GP + gl[l]
        per_core[c]["scat_rows"] = scat_rows
        scat = np.full(128, n_graphs, np.int64)
        scat[:int(Gc[c])] = glo[c] + np.arange(int(Gc[c]))
        per_core[c]["scat_g"] = scat.astype(np.int32)[:, None]

    meta = dict(N=N, NP=NP, LV=LV, T=T, NC=n_cores, BASE1=BASE1, BASE2=BASE2,
                chunks=chunks, sumJ=sumJ, n_graphs=n_graphs, dinv=dinv,
                GP=GP, Jp=Jp, ag_rows=ag_rows, perm=perm)
    return meta, per_core


# ---------------------------------------------------------------- bass build

def build(meta, CIN, HID, HMLP, NCL, n_queues=4):
    """Build the SPMD Bass program. All per-core variation flows via inputs."""
    m = meta
    T, NC = m["T"], m["NC"]
    N, NP, LV = m["N"], m["NP"], m["LV"]
    chunks = m["chunks"]
    GP, Jp, ag_rows = m["GP"], m["Jp"], m["ag_rows"]
    NG = m["n_graphs"]
    NGT = cdiv(NG, 128)          # pooled tiles (4)
    n_chunk_cols = sum(sum(w + 1 for w in cl) for cl in chunks)

    nc = bacc.Bacc("TRN2", target_bir_lowering=False, debug=False,
                   num_devices=NC, num_swdge_queues=n_queues)
    qctr = [0]

    def next_q():
        q = qctr[0] % n_queues
        qctr[0] += 1
        return q
    dt = mybir.dt

    # ---- inputs
    x_t = nc.dram_tensor("xs", [N + 1, CIN], BF16, kind="ExternalInput")
    xsp_t = nc.dram_tensor("xsp", [LV, CIN], BF16, kind="ExternalInput")
    idx1_t = nc.dram_tensor("idx1", [128, n_chunk_cols * 8], I16,
                            kind="ExternalInput")
    idx2_t = nc.dram_tensor("idx2", [128, n_chunk_cols * 8], I16,
                            kind="ExternalInput")
    dinvd_t = nc.dram_tensor("dinv_dst", [128, T], F32, kind="ExternalInput")
    scatr_t = nc.dram_tensor("scat_rows", [128, T], I32, kind="ExternalInput")
    scat_t = nc.dram_tensor("scat_g", [128, 1], I32, kind="ExternalInput")
    W1_t = nc.dram_tensor("W1", [CIN, HID], F32, kind="ExternalInput")
    b1_t = nc.dram_tensor("b1", [1, HID], F32, kind="ExternalInput")
    W2_t = nc.dram_tensor("W2", [HID, HID], F32, kind="ExternalInput")
    b2_t = nc.dram_tensor("b2", [1, HID], F32, kind="ExternalInput")
    fcW1_t = nc.dram_tensor("fcW1", [HID, HMLP], F32, kind="ExternalInput")
    fcb1_t = nc.dram_tensor("fcb1", [1, HMLP], F32, kind="ExternalInput")
    fcW2_t = nc.dram_tensor("fcW2", [HMLP, NCL], F32, kind="ExternalInput")
    fcb2_t = nc.dram_tensor("fcb2", [1, NCL], F32, kind="ExternalInput")
    out_t = nc.dram_tensor("out", [NG, NCL], F32, kind="ExternalOutput")

    KB1 = CIN // 128    # K blocks layer1 (1)
    KB2 = HID // 128    # K blocks layer2 (2)
    KBM = HID // 128    # fc1 K blocks (2)

    with tile.TileContext(nc) as tc:
        with (
            tc.tile_pool(name="const", bufs=1) as cpool,
            tc.tile_pool(name="gath", bufs=8) as gpool,
            tc.tile_pool(name="work", bufs=4) as wpool,
            tc.tile_pool(name="outp", bufs=3) as opool,
            tc.tile_pool(name="tp_ps", bufs=2, space="PSUM") as tp_ps,
            tc.tile_pool(name="mm_ps", bufs=2, space="PSUM") as mm_ps,
            tc.tile_pool(name="dram", bufs=1, space="DRAM") as dr,
        ):
            # ---- constants / weights to SBUF
            ident = cpool.tile([128, 128], F32)
            make_identity(nc, ident[:])
            ones = cpool.tile([1, 128], F32)
            nc.vector.memset(ones[:], 1.0)
            negbig = cpool.tile([128, HID], F32)
            nc.vector.memset(negbig[:], NEG_BIG)

            W1_sb = cpool.tile([128, KB1, HID], F32)
            for k in range(KB1):
                nc.sync.dma_start(out=W1_sb[:, k, :],
                                  in_=W1_t[k * 128:(k + 1) * 128, :])
            W2_sb = cpool.tile([128, KB2, HID], F32)
            for k in range(KB2):
                nc.sync.dma_start(out=W2_sb[:, k, :],
                                  in_=W2_t[k * 128:(k + 1) * 128, :])
            fcW1_sb = cpool.tile([128, KBM, HMLP], F32)
            for k in range(KBM):
                nc.sync.dma_start(out=fcW1_sb[:, k, :],
                                  in_=fcW1_t[k * 128:(k + 1) * 128, :])
            fcW2_sb = cpool.tile([128, NCL], F32)
            nc.sync.dma_start(out=fcW2_sb[:], in_=fcW2_t[:, :])
            b1_sb = cpool.tile([1, HID], F32)
            nc.sync.dma_start(out=b1_sb[:], in_=b1_t[:, :])
            b2_sb = cpool.tile([1, HID], F32)
            nc.sync.dma_start(out=b2_sb[:], in_=b2_t[:, :])
            fcb1_sb = cpool.tile([1, HMLP], F32)
            nc.sync.dma_start(out=fcb1_sb[:], in_=fcb1_t[:, :])
            fcb2_sb = cpool.tile([1, NCL], F32)
            nc.sync.dma_start(out=fcb2_sb[:], in_=fcb2_t[:, :])

            idx1_sb = cpool.tile([128, n_chunk_cols * 8], I16)
            nc.sync.dma_start(out=idx1_sb[:], in_=idx1_t[:, :])
            idx2_sb = cpool.tile([128, n_chunk_cols * 8], I16)
            nc.sync.dma_start(out=idx2_sb[:], in_=idx2_t[:, :])
            dinvd_sb = cpool.tile([128, T], F32)
            nc.sync.dma_start(out=dinvd_sb[:], in_=dinvd_t[:, :])
            scatr_sb = cpool.tile([128, T], I32)
            nc.sync.dma_start(out=scatr_sb[:], in_=scatr_t[:, :])
            scat_sb = cpool.tile([128, 1], I32)
            nc.sync.dma_start(out=scat_sb[:], in_=scat_t[:, :])

            # ---- internal DRAM
            h1_shard = dr.tile([LV, HID], BF16)
            h1_table = dr.tile([NP, HID], BF16)
            ag_out = []
            for agk, (r0, r1) in enumerate(ag_rows):
                agt = dr.tile([NC * (r1 - r0), HID], BF16,
                              addr_space="Shared", name=f"ag_out{agk}")
                ag_out.append(agt)
            JPG = (Jp + 1) * GP            # pool layout rows (+dump space)
            pool_layout = dr.tile([JPG, HID], F32)
            pool_scat = dr.tile([NG + 1, HID], F32)
            pool_red = dr.tile([NG, HID], F32, addr_space="Shared")

            # init pool layout + pool_scat table to NEG_BIG
            for i in range(cdiv(JPG, 128)):
                r0 = i * 128
                r1 = min(r0 + 128, JPG)
                nc.sync.dma_start(out=pool_layout[r0:r1, :],
                                  in_=negbig[0:r1 - r0, :])
            for i in range(cdiv(NG + 1, 128)):
                r0 = i * 128
                r1 = min(r0 + 128, NG + 1)
                nc.sync.dma_start(out=pool_scat[r0:r1, :],
                                  in_=negbig[0:r1 - r0, :])

            # chunked AllGather: chunk k fires once its h1 tiles are written
            ag_done = [False] * len(ag_rows)

            def fire_ag(k):
                r0, r1 = ag_rows[k]
                nc.gpsimd.collective_compute(
                    "AllGather", mybir.AluOpType.bypass,
                    replica_groups=[list(range(NC))],
                    ins=[h1_shard[r0:r1, :]],
                    outs=[ag_out[k][:, :]])
                ag_done[k] = True

            # ---------------- layer helper
            def gcn_layer(layer):
                if layer == 1:
                    C = CIN
                    idx_sb = idx1_sb
                    table_ap = x_t[:, :]
                    KB, W_sb, b_sb = KB1, W1_sb, b1_sb
                else:
                    C = HID
                    idx_sb = idx2_sb
                    table_ap = h1_table[:, :]
                    KB, W_sb, b_sb = KB2, W2_sb, b2_sb
                base = m["BASE1"] if layer == 1 else m["BASE2"]

                icol = 0   # idx column offset (units of 8 int16 per slot)
                gtag = "g1" if layer == 1 else "g2"
                for t in range(T):
                    acc = wpool.tile([128, HID], F32, tag="acc")
                    first = True
                    for w in chunks[t]:
                        g = gpool.tile([128, 8, C], BF16, tag=gtag)
                        nc.gpsimd.dma_gather(
                            g[:, 0:w + 1, 0:C],
                            table_ap[base:, :],
                            idx_sb[:, icol * 8:(icol + w + 1) * 8],
                            (w + 1) * 128, (w + 1) * 128, C,
                            queue_num=next_q())
                        # sum over the chunk's slots in one strided reduce
                        gv = g[:, 0:w, 0:C].rearrange("p j c -> p c j")
                        if first:
                            nc.vector.tensor_reduce(
                                out=acc[:, 0:C], in_=gv,
                                axis=mybir.AxisListType.X,
                                op=mybir.AluOpType.add)
                        else:
                            red = wpool.tile([128, HID], F32, tag="red")
                            nc.vector.tensor_reduce(
                                out=red[:, 0:C], in_=gv,
                                axis=mybir.AxisListType.X,
                                op=mybir.AluOpType.add)
                            nc.vector.tensor_add(
                                out=acc[:, 0:C], in0=acc[:, 0:C],
                                in1=red[:, 0:C])
                        first = False
                        icol += w + 1

                    # self-loop term: local (permuted-sequential) rows
                    sl = wpool.tile([128, HID], BF16, tag="self")
                    if layer == 1:
                        nc.sync.dma_start(
                            out=sl[:, 0:C],
                            in_=xsp_t[t * 128:(t + 1) * 128, :])
                    else:
                        nc.sync.dma_start(
                            out=sl[:, 0:C],
                            in_=h1_shard[t * 128:(t + 1) * 128, :])
                    nc.vector.tensor_add(out=acc[:, 0:C], in0=acc[:, 0:C],
                                         in1=sl[:, 0:C])

                    # dst-side dinv scaling (Scalar engine; DVE is loaded)
                    nc.scalar.activation(
                        out=acc[:, 0:C], in_=acc[:, 0:C],
                        func=mybir.ActivationFunctionType.Copy,
                        scale=dinvd_sb[:, t:t + 1])

                    # transpose -> lhsT blocks
                    accT = wpool.tile([128, KB, 128], F32, tag="accT")
                    for k in range(KB):
                        tps = tp_ps.tile([128, 128], F32, tag="tp")
                        nc.tensor.transpose(out=tps[:],
                                            in_=acc[:, k * 128:(k + 1) * 128],
                                            identity=ident[:])
                        nc.scalar.activation(
                            out=accT[:, k, :], in_=tps[:],
                            func=mybir.ActivationFunctionType.Copy)

                    # matmul: bias + sum_k accT_k.T @ W_k
                    mm = mm_ps.tile([128, HID], F32, tag="mm")
                    nc.tensor.matmul(out=mm[:], lhsT=ones[0:1, :],
                                     rhs=b_sb[0:1, :], start=True, stop=False)
                    for k in range(KB):
                        nc.tensor.matmul(out=mm[:], lhsT=accT[:, k, :],
                                         rhs=W_sb[:, k, :],
                                         start=False, stop=(k == KB - 1))

                    if layer == 1:
                        # h1' = relu(dinv * (aggW + b)) = dinv * relu(aggW+b)
                        h = opool.tile([128, HID], BF16, tag="h")
                        nc.scalar.activation(
                            out=h[:], in_=mm[:],
                            func=mybir.ActivationFunctionType.Relu,
                            scale=dinvd_sb[:, t:t + 1])
                        nc.sync.dma_start(
                            out=h1_shard[t * 128:(t + 1) * 128, :], in_=h[:])
                        # fire any AllGather chunk whose rows are written
                        # (pipeline lag: wait 3 tiles past the boundary)
                        for k, (r0, r1) in enumerate(ag_rows):
                            if not ag_done[k] and (t + 1) * 128 >= r1 + 640:
                                fire_ag(k)
                    else:
                        h = opool.tile([128, HID], F32, tag="h")
                        nc.scalar.activation(
                            out=h[:], in_=mm[:],
                            func=mybir.ActivationFunctionType.Relu)
                        # scatter rows into the pooling [j-slot, graph] layout
                        nc.gpsimd.indirect_dma_start(
                            out=pool_layout[:, :],
                            out_offset=bass.IndirectOffsetOnAxis(
                                ap=scatr_sb[:, t:t + 1], axis=0),
                            in_=h[:], in_offset=None)

            # ---------------- layer 1 + allgather
            gcn_layer(1)
            for k in range(len(ag_rows)):
                if not ag_done[k]:
                    fire_ag(k)
            # copies into the gather table run in the L1 pipeline-drain window
            for k, (r0, r1) in enumerate(ag_rows):
                nc.sync.dma_start(out=h1_table[NC * r0:NC * r1, :],
                                  in_=ag_out[k][:, :])

            # ---------------- layer 2
            gcn_layer(2)

            # ---------------- pooling: max over j-slots of the scatter layout
            pooled = wpool.tile([128, HID], F32, tag="pooled")
            nc.vector.memset(pooled[:], NEG_BIG)
            JC = 8
            for j0 in range(0, Jp, JC):
                jc = min(JC, Jp - j0)
                pt = gpool.tile([128, JC, HID], F32, tag="pool")
                dv = pool_layout[j0 * GP:(j0 + jc) * GP, :].rearrange(
                    "(j g) c -> g j c", j=jc)
                nc.sync.dma_start(out=pt[0:GP, 0:jc, :], in_=dv)
                red = wpool.tile([128, HID], F32, tag="red")
                pv = pt[0:GP, 0:jc, :].rearrange("g j c -> g c j")
                nc.vector.tensor_reduce(out=red[0:GP, :], in_=pv,
                                        axis=mybir.AxisListType.X,
                                        op=mybir.AluOpType.max)
                nc.vector.tensor_max(out=pooled[0:GP, :], in0=pooled[0:GP, :],
                                     in1=red[0:GP, :])
            nc.gpsimd.indirect_dma_start(
                out=pool_scat[:, :],
                out_offset=bass.IndirectOffsetOnAxis(ap=scat_sb[:, 0:1],
                                                     axis=0),
                in_=pooled[:],
                in_offset=None)
            nc.gpsimd.collective_compute(
                "AllReduce", mybir.AluOpType.max,
                replica_groups=[list(range(NC))],
                ins=[pool_scat[0:NG, :]], outs=[pool_red[:, :]])

            # ---------------- MLP + log_softmax (replicated)
            gT = wpool.tile([128, KBM, NGT * 128], F32, tag="gT")
            for i in range(NGT):
                gtile = wpool.tile([128, HID], F32, tag="gtile")
                r0, r1 = i * 128, min((i + 1) * 128, NG)
                if r1 - r0 < 128:
                    nc.vector.memset(gtile[:], 0.0)
                nc.sync.dma_start(out=gtile[0:r1 - r0, :],
                                  in_=pool_red[r0:r1, :])
                for k in range(KBM):
                    tps = tp_ps.tile([128, 128], F32, tag="tp")
                    nc.tensor.transpose(out=tps[:],
                                        in_=gtile[:, k * 128:(k + 1) * 128],
                                        identity=ident[:])
                    nc.vector.tensor_copy(out=gT[:, k, i * 128:(i + 1) * 128],
                                          in_=tps[:])
            o1T = wpool.tile([128, NGT * 128], F32, tag="o1T")
            for i in range(NGT):
                mm1 = mm_ps.tile([128, HMLP], F32, tag="mm")
                nc.tensor.matmul(out=mm1[:], lhsT=ones[0:1, :],
                                 rhs=fcb1_sb[0:1, :], start=True, stop=False)
                for k in range(KBM):
                    nc.tensor.matmul(out=mm1[:],
                                     lhsT=gT[:, k, i * 128:(i + 1) * 128],
                                     rhs=fcW1_sb[:, k, :],
                                     start=False, stop=(k == KBM - 1))
                o1 = wpool.tile([128, HMLP], F32, tag="o1")
                nc.scalar.activation(out=o1[:], in_=mm1[:],
                                     func=mybir.ActivationFunctionType.Relu)
                tps = tp_ps.tile([128, 128], F32, tag="tp")
                nc.tensor.transpose(out=tps[0:HMLP, :], in_=o1[:],
                                    identity=ident[:])
                nc.vector.tensor_copy(out=o1T[0:HMLP, i * 128:(i + 1) * 128],
                                      in_=tps[0:HMLP, :])
            for i in range(NGT):
                mm2 = mm_ps.tile([128, NCL], F32, tag="mm2")
                nc.tensor.matmul(out=mm2[:], lhsT=ones[0:1, :],
                                 rhs=fcb2_sb[0:1, :], start=True, stop=False)
                nc.tensor.matmul(out=mm2[:],
                                 lhsT=o1T[0:HMLP, i * 128:(i + 1) * 128],
                                 rhs=fcW2_sb[0:HMLP, :],
                                 start=False, stop=True)
                # log_softmax rows
                mx = wpool.tile([128, 1], F32, tag="mx")
                nc.vector.tensor_reduce(out=mx[:], in_=mm2[:],
                                        axis=mybir.AxisListType.X,
                                        op=mybir.AluOpType.max)
                tsh = wpool.tile([128, NCL], F32, tag="tsh")
                nc.vector.tensor_scalar(
                    out=tsh[:], in0=mm2[:], scalar1=mx[:, 0:1], scalar2=None,
                    op0=mybir.AluOpType.subtract)
                ex = wpool.tile([128, NCL], F32, tag="ex")
                nc.scalar.activation(out=ex[:], in_=tsh[:],
                                     func=mybir.ActivationFunctionType.Exp)
                sm = wpool.tile([128, 1], F32, tag="sm")
                nc.vector.tensor_reduce(out=sm[:], in_=ex[:],
                                        axis=mybir.AxisListType.X,
                                        op=mybir.AluOpType.add)
                ls = wpool.tile([128, 1], F32, tag="ls")
                nc.scalar.activation(out=ls[:], in_=sm[:],
                                     func=mybir.ActivationFunctionType.Ln)
                oo = opool.tile([128, NCL], F32, tag="oo")
                nc.vector.tensor_scalar(
                    out=oo[:], in0=tsh[:], scalar1=ls[:, 0:1], scalar2=None,
                    op0=mybir.AluOpType.subtract)
                r0, r1 = i * 128, min((i + 1) * 128, NG)
                nc.sync.dma_start(out=out_t[r0:r1, :], in_=oo[0:r1 - r0, :])

    nc.compile()
    return nc


# ---------------------------------------------------------------- entry

def _ensure_ntff_hook():
    """Install the axon NTFF profile hook if the image's antenv lacks it.

    Dev-only (trace=True): lets run_bass_kernel_spmd return exec_time_ns.
    """
    import sys as _sys
    import types as _types
    try:
        from antenv.axon_hooks import get_axon_ntff_profile_hook  # noqa
        return
    except ImportError:
        pass
    try:
        _sys.path.insert(0, "/root/.axon_site")
        from trn_agent_boot.trn_boot import _ntff_profile_via_ctypes
        hook = _ntff_profile_via_ctypes("/opt/axon/libaxon_pjrt.so")
        mod = _types.ModuleType("antenv.axon_hooks")
        mod._hook = hook
        mod.get_axon_ntff_profile_hook = lambda: mod._hook
        mod.set_axon_ntff_profile_hook = lambda h: setattr(mod, "_hook", h)
        _sys.modules["antenv.axon_hooks"] = mod
        # artifact upload needs a bucket; degrade to no-op on failure
        _orig_upload = bass_utils.upload_artifacts

        def _safe_upload(tmpdir):
            try:
                return _orig_upload(tmpdir)
            except Exception:
                return tmpdir
        bass_utils.upload_artifacts = _safe_upload
    except Exception:
        pass


def kernel(x, edge_index, batch, W1, b1, W2, b2, fcW1, fcb1, fcW2, fcb2,
           trace=False):
    if trace:
        _ensure_ntff_hook()
    x = np.asarray(x, np.float32)
    edge_index = np.asarray(edge_index, np.int64)
    batch = np.asarray(batch, np.int64)
    NG = 512
    meta, per_core = prep(x, edge_index, batch, NG)

    nc = build(meta, CIN=x.shape[1], HID=W1.shape[1], HMLP=fcW1.shape[1],
               NCL=fcW2.shape[1])

    import ml_dtypes
    xs = np.concatenate([meta["dinv"][:, None] * x,
                         np.zeros((1, x.shape[1]), np.float32)], axis=0)
    xs_bf = xs.astype(ml_dtypes.bfloat16)
    shared = dict(
        xs=xs_bf,
        W1=np.asarray(W1, np.float32), b1=np.asarray(b1, np.float32)[None, :],
        W2=np.asarray(W2, np.float32), b2=np.asarray(b2, np.float32)[None, :],
        fcW1=np.asarray(fcW1, np.float32),
        fcb1=np.asarray(fcb1, np.float32)[None, :],
        fcW2=np.asarray(fcW2, np.float32),
        fcb2=np.asarray(fcb2, np.float32)[None, :],
    )
    perm = meta["perm"]
    in_maps = []
    for c in range(meta["NC"]):
        d = dict(shared)
        pc = per_core[c]
        # permuted local xs rows (pads -> zero row N)
        pidx = np.where(perm[c] >= 0, perm[c], x.shape[0])
        d["xsp"] = xs_bf[pidx]
        d["idx1"] = pc["idx1"]
        d["idx2"] = pc["idx2"]
        d["dinv_dst"] = pc["dinv_dst"]
        d["scat_rows"] = pc["scat_rows"]
        d["scat_g"] = np.tile(pc["scat_g"], (1, 1))
        in_maps.append(d)

    res = bass_utils.run_bass_kernel_spmd(
        nc, in_maps, core_ids=list(range(meta["NC"])), trace=trace)
    out = res.results[0]["out"]
    kernel.last_exec_time_ns = res.exec_time_ns
    return out


kernel.last_exec_time_ns = None



# revision 32
# speedup vs baseline: 2.1923x; 1.0285x over previous
"""GCN (2-layer GCNConv + global max pool + MLP + log_softmax) on 8 trn2 cores.

Strategy (sharding_hint: partition nodes + incident edges, replicate weights):
  - Nodes are partitioned 6250/core (+22 pad nodes/core -> 6272 = 49 tiles of
    128). Within each core, nodes are sorted by degree (desc) so that the
    per-tile padded gather width J_t ~= the true degree.
  - Edges are grouped by dst; each core owns edges into its nodes. For each
    128-node tile the messages are fetched with dma_gather (int16 signed
    indices relative to a mid-table base row cover all 50176/50000 rows),
    giving [128 nodes, J, C] tiles which are reduced on DVE.
  - GCN normalization: agg = D^-1/2 (A+I) D^-1/2 h. Layer-1 folds
    dinv[src] into the DVE accumulate; the produced h1 is pre-scaled by
    dinv (h1' = dinv * relu(...)), so layer-2 accumulation is plain adds.
  - h1' shards are AllGathered into a replicated table; layer-2 gathers
    from it. Max pooling is a dma_gather per graph-partition from the local
    h2 shard + DVE max-reduce, scattered into a [513,256] table (indirect
    scatter handles per-core graph offsets), AllReduce(max), then the small
    MLP + log_softmax run replicated on every core.
"""

import numpy as np

import concourse.bass as bass
import concourse.bacc as bacc
import concourse.tile as tile
import concourse.mybir as mybir
from concourse import bass_utils
from concourse.masks import make_identity
from concourse._compat import cdiv

F32 = mybir.dt.float32
BF16 = mybir.dt.bfloat16
I16 = mybir.dt.int16
I32 = mybir.dt.int32

NEG_BIG = -1.0e38


# ---------------------------------------------------------------- host prep

def _wrap_idx(flat):
    """j-major flat int16 idx list [n] -> wrapped SBUF layout [128, n//16].

    dma_gather consumes idx i from wrapped[i % 16, i // 16]; the 16-row
    pattern is replicated to all 128 partitions.
    """
    n = len(flat)
    assert n % 128 == 0
    w = np.zeros((16, n // 16), np.int16)
    w[np.arange(n) % 16, np.arange(n) // 16] = flat
    return np.tile(w, (8, 1))


def prep(x, edge_index, batch, n_graphs, n_cores=8, j_cap=6, n_ag_chunks=4,
         mid_base=True):
    """All index-space preprocessing. Returns (meta, per-core arrays).

    - The x table fed to layer 1 is host-prescaled (xs = dinv * x) with a
      trailing zero row at index N, so pad slots gather exact zeros and the
      on-device accumulate is a plain sum for both layers.
    - The reference's added self-loops are NOT emitted as gather slots; the
      kernel adds the local (permuted-sequential) row per tile instead.
    - The h1 table is laid out AllGather-chunk-major: chunk k holds rows
      [NC * r0_k, NC * r1_k) as [core][local row] so each chunked AllGather
      writes a contiguous range.
    """
    N = x.shape[0]
    NR = N // n_cores                      # real nodes per core
    LV = int(cdiv(NR, 128)) * 128          # padded nodes per core
    T = LV // 128                          # tiles per core
    NP = LV * n_cores                      # padded total
    BASE1 = (N + 1) // 2 if mid_base else 0   # xs-table base row
    BASE2 = NP // 2 if mid_base else 0     # h1-table base row
    assert max(N + 1 - BASE1, BASE1, NP - BASE2, BASE2, NR + 1) <= 32767

    src_e = np.asarray(edge_index[0])
    dst_e = np.asarray(edge_index[1])
    deg = np.bincount(dst_e, minlength=N).astype(np.int64) + 1  # + self-loop
    dinv = (1.0 / np.sqrt(deg.astype(np.float32))).astype(np.float32)

    # group non-self edges by dst
    order = np.argsort(dst_e, kind="stable")
    src_s = src_e[order]
    starts = np.searchsorted(dst_e[order], np.arange(N))
    ends = np.searchsorted(dst_e[order], np.arange(N) + 1)

    # per-core degree-sorted permutation; perm[c][l] = orig id, -1 = pad
    perm = np.full((n_cores, LV), -1, np.int64)
    for c in range(n_cores):
        lo = NR * c
        perm[c, :NR] = np.argsort(-deg[lo:lo + NR], kind="stable") + lo

    # AllGather chunk boundaries (in tiles -> local rows); smaller last
    # chunk so the post-L1 exposure is short
    if n_ag_chunks == 4 and T == 49:
        bt = [0, 15, 29, 42, 49]
    else:
        bt = [round(k * T / n_ag_chunks) for k in range(n_ag_chunks + 1)]
    ag_rows = [(bt[k] * 128, bt[k + 1] * 128) for k in range(n_ag_chunks)]
    # orig id -> h1-table row (chunk-major AllGather layout)
    row_of = np.zeros(LV, np.int64)        # local row -> table row offset fn
    for (r0, r1) in ag_rows:
        row_of[r0:r1] = n_cores * r0 + np.arange(r1 - r0)
    chunk_len = np.zeros(LV, np.int64)
    for (r0, r1) in ag_rows:
        chunk_len[r0:r1] = r1 - r0
    perm_row = np.full(N, -1, np.int64)   # orig id -> h1-table row
    for c in range(n_cores):
        loc = np.arange(NR)
        perm_row[perm[c, :NR]] = row_of[loc] + c * chunk_len[loc]

    # J_t per tile (max over cores) of NON-SELF in-degree
    nsd = ends - starts
    Jt = np.zeros(T, np.int64)
    for c in range(n_cores):
        for t in range(T):
            ids = perm[c, t * 128:(t + 1) * 128]
            ids = ids[ids >= 0]
            if len(ids):
                Jt[t] = max(Jt[t], nsd[ids].max())
    Jt = np.maximum(Jt, 1)
    chunks = [[j_cap] * (int(j) // j_cap) + ([int(j) % j_cap] if j % j_cap else [])
              for j in Jt]
    sumJ = int(Jt.sum())

    # pad rows: any pad node's permuted row (h1' there is forced to 0);
    # the globally-last pad row lands at table row NP - 1 in every layout.
    pad_row2 = NP - 1 if NP > N else None
    assert pad_row2 is not None, "need at least one pad node for L2 padding"

    # per-core slot tables
    per_core = []
    for c in range(n_cores):
        idx1 = np.zeros((sumJ * 128,), np.int16)
        idx2 = np.zeros((sumJ * 128,), np.int16)
        dinv_dst = np.zeros((128, T), np.float32)
        off = 0
        for t in range(T):
            J = int(Jt[t])
            for p in range(128):
                n = perm[c, t * 128 + p]
                if n >= 0:
                    dinv_dst[p, t] = dinv[n]
                    ss = src_s[starts[n]:ends[n]]
                    nj = len(ss)
                    sl = (off + np.arange(nj)) * 128 + p
                    idx1[sl] = (ss - BASE1).astype(np.int16)
                    idx2[sl] = (perm_row[ss] - BASE2).astype(np.int16)
                else:
                    nj = 0
                # pad slots gather exact-zero rows
                if nj < J:
                    sl = (off + np.arange(nj, J)) * 128 + p
                    idx1[sl] = N - BASE1          # zero row of xs
                    idx2[sl] = pad_row2 - BASE2   # zero row
            off += J
        assert off == sumJ

        # wrapped layout per gather chunk; each chunk gets one trailing
        # all-pad block: the gather's final descriptor flakily skips its
        # data write, so the last 128 slots are sacrificial and never read.
        pad_blk = np.zeros(128, np.int16)
        w1 = []
        w2 = []
        off = 0
        for t in range(T):
            for w in chunks[t]:
                blk = slice(off * 128, (off + w) * 128)
                w1.append(_wrap_idx(np.concatenate([idx1[blk], pad_blk])))
                w2.append(_wrap_idx(np.concatenate([idx2[blk], pad_blk])))
                off += w
        idx1_w = np.concatenate(w1, axis=1)
        idx2_w = np.concatenate(w2, axis=1)
        per_core.append(dict(idx1=idx1_w, idx2=idx2_w, dinv_dst=dinv_dst))

    # pooling: h2 rows are scattered into a [j-slot, local graph] layout,
    # then max-reduced over j-slots.  GP/Jp are maxed over cores (SPMD).
    glo = np.zeros(n_cores, np.int64)
    Gc = np.zeros(n_cores, np.int64)
    for c in range(n_cores):
        b = batch[NR * c:NR * (c + 1)]
        glo[c] = b.min()
        Gc[c] = b.max() - b.min() + 1
    GP = int(Gc.max())
    assert GP <= 128
    # member slot j for each local permuted row
    jslot = []
    Jp = 0
    for c in range(n_cores):
        b = batch[NR * c:NR * (c + 1)]
        cnt = np.zeros(GP, np.int64)
        js = np.full(LV, -1, np.int64)
        gl = np.full(LV, -1, np.int64)
        for l in range(LV):
            node = perm[c, l]
            if node >= 0:
                g = int(batch[node] - glo[c])
                js[l] = cnt[g]
                gl[l] = g
                cnt[g] += 1
        jslot.append((js, gl))
        Jp = max(Jp, int(cnt.max()))
    for c in range(n_cores):
        js, gl = jslot[c]
        scat_rows = np.full((128, T), Jp * GP, np.int32)  # dump row
        for l in range(LV):
            if js[l] >= 0:
                scat_rows[l % 128, l // 128] = js[l] * GP + gl[l]
        per_core[c]["scat_rows"] = scat_rows
        scat = np.full(128, n_graphs, np.int64)
        scat[:int(Gc[c])] = glo[c] + np.arange(int(Gc[c]))
        per_core[c]["scat_g"] = scat.astype(np.int32)[:, None]

    meta = dict(N=N, NP=NP, LV=LV, T=T, NC=n_cores, BASE1=BASE1, BASE2=BASE2,
                chunks=chunks, sumJ=sumJ, n_graphs=n_graphs, dinv=dinv,
                GP=GP, Jp=Jp, ag_rows=ag_rows, perm=perm)
    return meta, per_core


# ---------------------------------------------------------------- bass build

def build(meta, CIN, HID, HMLP, NCL, n_queues=4):
    """Build the SPMD Bass program. All per-core variation flows via inputs."""
    m = meta
    T, NC = m["T"], m["NC"]
    N, NP, LV = m["N"], m["NP"], m["LV"]
    chunks = m["chunks"]
    GP, Jp, ag_rows = m["GP"], m["Jp"], m["ag_rows"]
    NG = m["n_graphs"]
    NGT = cdiv(NG, 128)          # pooled tiles (4)
    n_chunk_cols = sum(sum(w + 1 for w in cl) for cl in chunks)

    nc = bacc.Bacc("TRN2", target_bir_lowering=False, debug=False,
                   num_devices=NC, num_swdge_queues=n_queues)
    qctr = [0]

    def next_q(avoid0=False):
        if avoid0:
            q = 1 + qctr[0] % (n_queues - 1)
        else:
            q = qctr[0] % n_queues
        qctr[0] += 1
        return q
    dt = mybir.dt

    # ---- inputs
    x_t = nc.dram_tensor("xs", [N + 1, CIN], BF16, kind="ExternalInput")
    xsp_t = nc.dram_tensor("xsp", [LV, CIN], BF16, kind="ExternalInput")
    idx1_t = nc.dram_tensor("idx1", [128, n_chunk_cols * 8], I16,
                            kind="ExternalInput")
    idx2_t = nc.dram_tensor("idx2", [128, n_chunk_cols * 8], I16,
                            kind="ExternalInput")
    dinvd_t = nc.dram_tensor("dinv_dst", [128, T], F32, kind="ExternalInput")
    scatr_t = nc.dram_tensor("scat_rows", [128, T], I32, kind="ExternalInput")
    scat_t = nc.dram_tensor("scat_g", [128, 1], I32, kind="ExternalInput")
    W1_t = nc.dram_tensor("W1", [CIN, HID], F32, kind="ExternalInput")
    b1_t = nc.dram_tensor("b1", [1, HID], F32, kind="ExternalInput")
    W2_t = nc.dram_tensor("W2", [HID, HID], F32, kind="ExternalInput")
    b2_t = nc.dram_tensor("b2", [1, HID], F32, kind="ExternalInput")
    fcW1_t = nc.dram_tensor("fcW1", [HID, HMLP], F32, kind="ExternalInput")
    fcb1_t = nc.dram_tensor("fcb1", [1, HMLP], F32, kind="ExternalInput")
    fcW2_t = nc.dram_tensor("fcW2", [HMLP, NCL], F32, kind="ExternalInput")
    fcb2_t = nc.dram_tensor("fcb2", [1, NCL], F32, kind="ExternalInput")
    out_t = nc.dram_tensor("out", [NG, NCL], F32, kind="ExternalOutput")

    KB1 = CIN // 128    # K blocks layer1 (1)
    KB2 = HID // 128    # K blocks layer2 (2)
    KBM = HID // 128    # fc1 K blocks (2)

    with tile.TileContext(nc) as tc:
        with (
            tc.tile_pool(name="const", bufs=1) as cpool,
            tc.tile_pool(name="gath", bufs=6) as gpool,
            tc.tile_pool(name="poolld", bufs=2) as ppool,
            tc.tile_pool(name="work", bufs=4) as wpool,
            tc.tile_pool(name="outp", bufs=3) as opool,
            tc.tile_pool(name="tp_ps", bufs=2, space="PSUM") as tp_ps,
            tc.tile_pool(name="mm_ps", bufs=2, space="PSUM") as mm_ps,
            tc.tile_pool(name="dram", bufs=1, space="DRAM") as dr,
        ):
            # ---- constants / weights to SBUF
            ident = cpool.tile([128, 128], F32)
            make_identity(nc, ident[:])
            ones = cpool.tile([1, 128], F32)
            nc.vector.memset(ones[:], 1.0)
            negbig = cpool.tile([128, HID], BF16)
            nc.vector.memset(negbig[:], NEG_BIG)

            W1_sb = cpool.tile([128, KB1, HID], F32)
            for k in range(KB1):
                nc.sync.dma_start(out=W1_sb[:, k, :],
                                  in_=W1_t[k * 128:(k + 1) * 128, :])
            W2_sb = cpool.tile([128, KB2, HID], F32)
            for k in range(KB2):
                nc.sync.dma_start(out=W2_sb[:, k, :],
                                  in_=W2_t[k * 128:(k + 1) * 128, :])
            fcW1_sb = cpool.tile([128, KBM, HMLP], F32)
            for k in range(KBM):
                nc.sync.dma_start(out=fcW1_sb[:, k, :],
                                  in_=fcW1_t[k * 128:(k + 1) * 128, :])
            fcW2_sb = cpool.tile([128, NCL], F32)
            nc.sync.dma_start(out=fcW2_sb[:], in_=fcW2_t[:, :])
            b1_sb = cpool.tile([1, HID], F32)
            nc.sync.dma_start(out=b1_sb[:], in_=b1_t[:, :])
            b2_sb = cpool.tile([1, HID], F32)
            nc.sync.dma_start(out=b2_sb[:], in_=b2_t[:, :])
            fcb1_sb = cpool.tile([1, HMLP], F32)
            nc.sync.dma_start(out=fcb1_sb[:], in_=fcb1_t[:, :])
            fcb2_sb = cpool.tile([1, NCL], F32)
            nc.sync.dma_start(out=fcb2_sb[:], in_=fcb2_t[:, :])

            idx1_sb = cpool.tile([128, n_chunk_cols * 8], I16)
            nc.sync.dma_start(out=idx1_sb[:], in_=idx1_t[:, :])
            idx2_sb = cpool.tile([128, n_chunk_cols * 8], I16)
            nc.sync.dma_start(out=idx2_sb[:], in_=idx2_t[:, :])
            dinvd_sb = cpool.tile([128, T], F32)
            nc.sync.dma_start(out=dinvd_sb[:], in_=dinvd_t[:, :])
            scatr_sb = cpool.tile([128, T], I32)
            nc.sync.dma_start(out=scatr_sb[:], in_=scatr_t[:, :])
            scat_sb = cpool.tile([128, 1], I32)
            nc.sync.dma_start(out=scat_sb[:], in_=scat_t[:, :])

            # ---- internal DRAM
            h1_shard = dr.tile([LV, HID], BF16)
            h1_table = dr.tile([NP, HID], BF16)
            ag_out = []
            for agk, (r0, r1) in enumerate(ag_rows):
                agt = dr.tile([NC * (r1 - r0), HID], BF16,
                              addr_space="Shared", name=f"ag_out{agk}")
                ag_out.append(agt)
            JPG = (Jp + 1) * GP            # pool layout rows (+dump space)
            pool_layout = dr.tile([JPG, HID], BF16)
            pool_scat = dr.tile([NG + 1, HID], BF16)
            pool_red = dr.tile([NG, HID], BF16, addr_space="Shared")

            # init pool layout + pool_scat table to NEG_BIG
            for i in range(cdiv(JPG, 128)):
                r0 = i * 128
                r1 = min(r0 + 128, JPG)
                nc.sync.dma_start(out=pool_layout[r0:r1, :],
                                  in_=negbig[0:r1 - r0, :])
            for i in range(cdiv(NG + 1, 128)):
                r0 = i * 128
                r1 = min(r0 + 128, NG + 1)
                nc.sync.dma_start(out=pool_scat[r0:r1, :],
                                  in_=negbig[0:r1 - r0, :])

            # chunked AllGather: chunk k fires once its h1 tiles are written
            ag_done = [False] * len(ag_rows)

            def fire_ag(k):
                r0, r1 = ag_rows[k]
                nc.gpsimd.collective_compute(
                    "AllGather", mybir.AluOpType.bypass,
                    replica_groups=[list(range(NC))],
                    ins=[h1_shard[r0:r1, :]],
                    outs=[ag_out[k][:, :]])
                nc.sync.dma_start(out=h1_table[NC * r0:NC * r1, :],
                                  in_=ag_out[k][:, :])
                ag_done[k] = True

            # ---------------- layer helper
            def gcn_layer(layer):
                if layer == 1:
                    C = CIN
                    idx_sb = idx1_sb
                    table_ap = x_t[:, :]
                    KB, W_sb, b_sb = KB1, W1_sb, b1_sb
                else:
                    C = HID
                    idx_sb = idx2_sb
                    table_ap = h1_table[:, :]
                    KB, W_sb, b_sb = KB2, W2_sb, b2_sb
                base = m["BASE1"] if layer == 1 else m["BASE2"]

                icol = 0   # idx column offset (units of 8 int16 per slot)
                gtag = "g1" if layer == 1 else "g2"
                for t in range(T):
                    acc = wpool.tile([128, HID], F32, tag="acc")
                    first = True
                    for w in chunks[t]:
                        g = gpool.tile([128, 8, C], BF16, tag=gtag)
                        nc.gpsimd.dma_gather(
                            g[:, 0:w + 1, 0:C],
                            table_ap[base:, :],
                            idx_sb[:, icol * 8:(icol + w + 1) * 8],
                            (w + 1) * 128, (w + 1) * 128, C,
                            queue_num=next_q(avoid0=(layer == 2)))
                        # sum over the chunk's slots in one strided reduce
                        gv = g[:, 0:w, 0:C].rearrange("p j c -> p c j")
                        if first:
                            nc.vector.tensor_reduce(
                                out=acc[:, 0:C], in_=gv,
                                axis=mybir.AxisListType.X,
                                op=mybir.AluOpType.add)
                        else:
                            red = wpool.tile([128, HID], F32, tag="red")
                            nc.vector.tensor_reduce(
                                out=red[:, 0:C], in_=gv,
                                axis=mybir.AxisListType.X,
                                op=mybir.AluOpType.add)
                            nc.vector.tensor_add(
                                out=acc[:, 0:C], in0=acc[:, 0:C],
                                in1=red[:, 0:C])
                        first = False
                        icol += w + 1

                    # self-loop term: local (permuted-sequential) rows
                    sl = wpool.tile([128, HID], BF16, tag="self")
                    if layer == 1:
                        nc.sync.dma_start(
                            out=sl[:, 0:C],
                            in_=xsp_t[t * 128:(t + 1) * 128, :])
                    else:
                        nc.sync.dma_start(
                            out=sl[:, 0:C],
                            in_=h1_shard[t * 128:(t + 1) * 128, :])
                    nc.vector.tensor_add(out=acc[:, 0:C], in0=acc[:, 0:C],
                                         in1=sl[:, 0:C])

                    # dst-side dinv scaling (Scalar engine; DVE is loaded)
                    nc.scalar.activation(
                        out=acc[:, 0:C], in_=acc[:, 0:C],
                        func=mybir.ActivationFunctionType.Copy,
                        scale=dinvd_sb[:, t:t + 1])

                    # transpose -> lhsT blocks
                    accT = wpool.tile([128, KB, 128], F32, tag="accT")
                    for k in range(KB):
                        tps = tp_ps.tile([128, 128], F32, tag="tp")
                        nc.tensor.transpose(out=tps[:],
                                            in_=acc[:, k * 128:(k + 1) * 128],
                                            identity=ident[:])
                        nc.scalar.activation(
                            out=accT[:, k, :], in_=tps[:],
                            func=mybir.ActivationFunctionType.Copy)

                    # matmul: bias + sum_k accT_k.T @ W_k
                    mm = mm_ps.tile([128, HID], F32, tag="mm")
                    nc.tensor.matmul(out=mm[:], lhsT=ones[0:1, :],
                                     rhs=b_sb[0:1, :], start=True, stop=False)
                    for k in range(KB):
                        nc.tensor.matmul(out=mm[:], lhsT=accT[:, k, :],
                                         rhs=W_sb[:, k, :],
                                         start=False, stop=(k == KB - 1))

                    if layer == 1:
                        # h1' = relu(dinv * (aggW + b)) = dinv * relu(aggW+b)
                        h = opool.tile([128, HID], BF16, tag="h")
                        nc.scalar.activation(
                            out=h[:], in_=mm[:],
                            func=mybir.ActivationFunctionType.Relu,
                            scale=dinvd_sb[:, t:t + 1])
                        nc.sync.dma_start(
                            out=h1_shard[t * 128:(t + 1) * 128, :], in_=h[:])
                        # fire any AllGather chunk whose rows are written
                        # (pipeline lag: wait 3 tiles past the boundary)
                        for k, (r0, r1) in enumerate(ag_rows):
                            if not ag_done[k] and (t + 1) * 128 >= r1 + 1536:
                                fire_ag(k)
                    else:
                        h = opool.tile([128, HID], BF16, tag="h")
                        nc.scalar.activation(
                            out=h[:], in_=mm[:],
                            func=mybir.ActivationFunctionType.Relu)
                        # scatter rows into the pooling [j-slot, graph] layout
                        nc.gpsimd.indirect_dma_start(
                            out=pool_layout[:, :],
                            out_offset=bass.IndirectOffsetOnAxis(
                                ap=scatr_sb[:, t:t + 1], axis=0),
                            in_=h[:], in_offset=None)

            # ---------------- layer 1 + allgather
            gcn_layer(1)
            for k in range(len(ag_rows)):
                if not ag_done[k]:
                    fire_ag(k)

            # ---------------- layer 2
            gcn_layer(2)

            # ---------------- pooling: max over j-slots of the scatter layout
            pooled = wpool.tile([128, HID], BF16, tag="pooled")
            nc.vector.memset(pooled[:], NEG_BIG)
            JC = 16
            for j0 in range(0, Jp, JC):
                jc = min(JC, Jp - j0)
                pt = ppool.tile([128, JC, HID], BF16, tag="pool")
                dv = pool_layout[j0 * GP:(j0 + jc) * GP, :].rearrange(
                    "(j g) c -> g j c", j=jc)
                nc.sync.dma_start(out=pt[0:GP, 0:jc, :], in_=dv)
                red = wpool.tile([128, HID], BF16, tag="red2")
                pv = pt[0:GP, 0:jc, :].rearrange("g j c -> g c j")
                nc.vector.tensor_reduce(out=red[0:GP, :], in_=pv,
                                        axis=mybir.AxisListType.X,
                                        op=mybir.AluOpType.max)
                nc.vector.tensor_max(out=pooled[0:GP, :], in0=pooled[0:GP, :],
                                     in1=red[0:GP, :])
            nc.gpsimd.indirect_dma_start(
                out=pool_scat[:, :],
                out_offset=bass.IndirectOffsetOnAxis(ap=scat_sb[:, 0:1],
                                                     axis=0),
                in_=pooled[:],
                in_offset=None)
            nc.gpsimd.collective_compute(
                "AllReduce", mybir.AluOpType.max,
                replica_groups=[list(range(NC))],
                ins=[pool_scat[0:NG, :]], outs=[pool_red[:, :]])

            # ---------------- MLP + log_softmax (replicated)
            gT = wpool.tile([128, KBM, NGT * 128], F32, tag="gT")
            for i in range(NGT):
                gtile_bf = wpool.tile([128, HID], BF16, tag="gtileb")
                gtile = wpool.tile([128, HID], F32, tag="gtile")
                r0, r1 = i * 128, min((i + 1) * 128, NG)
                if r1 - r0 < 128:
                    nc.vector.memset(gtile[:], 0.0)
                nc.sync.dma_start(out=gtile_bf[0:r1 - r0, :],
                                  in_=pool_red[r0:r1, :])
                nc.vector.tensor_copy(out=gtile[0:r1 - r0, :],
                                      in_=gtile_bf[0:r1 - r0, :])
                for k in range(KBM):
                    tps = tp_ps.tile([128, 128], F32, tag="tp")
                    nc.tensor.transpose(out=tps[:],
                                        in_=gtile[:, k * 128:(k + 1) * 128],
                                        identity=ident[:])
                    nc.vector.tensor_copy(out=gT[:, k, i * 128:(i + 1) * 128],
                                          in_=tps[:])
            o1T = wpool.tile([128, NGT * 128], F32, tag="o1T")
            for i in range(NGT):
                mm1 = mm_ps.tile([128, HMLP], F32, tag="mm")
                nc.tensor.matmul(out=mm1[:], lhsT=ones[0:1, :],
                                 rhs=fcb1_sb[0:1, :], start=True, stop=False)
                for k in range(KBM):
                    nc.tensor.matmul(out=mm1[:],
                                     lhsT=gT[:, k, i * 128:(i + 1) * 128],
                                     rhs=fcW1_sb[:, k, :],
                                     start=False, stop=(k == KBM - 1))
                o1 = wpool.tile([128, HMLP], F32, tag="o1")
                nc.scalar.activation(out=o1[:], in_=mm1[:],
                                     func=mybir.ActivationFunctionType.Relu)
                tps = tp_ps.tile([128, 128], F32, tag="tp")
                nc.tensor.transpose(out=tps[0:HMLP, :], in_=o1[:],
                                    identity=ident[:])
                nc.vector.tensor_copy(out=o1T[0:HMLP, i * 128:(i + 1) * 128],
                                      in_=tps[0:HMLP, :])
            for i in range(NGT):
                mm2 = mm_ps.tile([128, NCL], F32, tag="mm2")
                nc.tensor.matmul(out=mm2[:], lhsT=ones[0:1, :],
                                 rhs=fcb2_sb[0:1, :], start=True, stop=False)
                nc.tensor.matmul(out=mm2[:],
                                 lhsT=o1T[0:HMLP, i * 128:(i + 1) * 128],
                                 rhs=fcW2_sb[0:HMLP, :],
                                 start=False, stop=True)
                # log_softmax rows
                mx = wpool.tile([128, 1], F32, tag="mx")
                nc.vector.tensor_reduce(out=mx[:], in_=mm2[:],
                                        axis=mybir.AxisListType.X,
                                        op=mybir.AluOpType.max)
                tsh = wpool.tile([128, NCL], F32, tag="tsh")
                nc.vector.tensor_scalar(
                    out=tsh[:], in0=mm2[:], scalar1=mx[:, 0:1], scalar2=None,
                    op0=mybir.AluOpType.subtract)
                ex = wpool.tile([128, NCL], F32, tag="ex")
                nc.scalar.activation(out=ex[:], in_=tsh[:],
                                     func=mybir.ActivationFunctionType.Exp)
                sm = wpool.tile([128, 1], F32, tag="sm")
                nc.vector.tensor_reduce(out=sm[:], in_=ex[:],
                                        axis=mybir.AxisListType.X,
                                        op=mybir.AluOpType.add)
                ls = wpool.tile([128, 1], F32, tag="ls")
                nc.scalar.activation(out=ls[:], in_=sm[:],
                                     func=mybir.ActivationFunctionType.Ln)
                oo = opool.tile([128, NCL], F32, tag="oo")
                nc.vector.tensor_scalar(
                    out=oo[:], in0=tsh[:], scalar1=ls[:, 0:1], scalar2=None,
                    op0=mybir.AluOpType.subtract)
                r0, r1 = i * 128, min((i + 1) * 128, NG)
                nc.sync.dma_start(out=out_t[r0:r1, :], in_=oo[0:r1 - r0, :])

    nc.compile()
    return nc


# ---------------------------------------------------------------- entry

def _ensure_ntff_hook():
    """Install the axon NTFF profile hook if the image's antenv lacks it.

    Dev-only (trace=True): lets run_bass_kernel_spmd return exec_time_ns.
    """
    import sys as _sys
    import types as _types
    try:
        from antenv.axon_hooks import get_axon_ntff_profile_hook  # noqa
        return
    except ImportError:
        pass
    try:
        _sys.path.insert(0, "/root/.axon_site")
        from trn_agent_boot.trn_boot import _ntff_profile_via_ctypes
        hook = _ntff_profile_via_ctypes("/opt/axon/libaxon_pjrt.so")
        mod = _types.ModuleType("antenv.axon_hooks")
        mod._hook = hook
        mod.get_axon_ntff_profile_hook = lambda: mod._hook
        mod.set_axon_ntff_profile_hook = lambda h: setattr(mod, "_hook", h)
        _sys.modules["antenv.axon_hooks"] = mod
        # artifact upload needs a bucket; degrade to no-op on failure
        _orig_upload = bass_utils.upload_artifacts

        def _safe_upload(tmpdir):
            try:
                return _orig_upload(tmpdir)
            except Exception:
                return tmpdir
        bass_utils.upload_artifacts = _safe_upload
    except Exception:
        pass


def kernel(x, edge_index, batch, W1, b1, W2, b2, fcW1, fcb1, fcW2, fcb2,
           trace=False):
    if trace:
        _ensure_ntff_hook()
    x = np.asarray(x, np.float32)
    edge_index = np.asarray(edge_index, np.int64)
    batch = np.asarray(batch, np.int64)
    NG = 512
    meta, per_core = prep(x, edge_index, batch, NG)

    nc = build(meta, CIN=x.shape[1], HID=W1.shape[1], HMLP=fcW1.shape[1],
               NCL=fcW2.shape[1])

    import ml_dtypes
    xs = np.concatenate([meta["dinv"][:, None] * x,
                         np.zeros((1, x.shape[1]), np.float32)], axis=0)
    xs_bf = xs.astype(ml_dtypes.bfloat16)
    shared = dict(
        xs=xs_bf,
        W1=np.asarray(W1, np.float32), b1=np.asarray(b1, np.float32)[None, :],
        W2=np.asarray(W2, np.float32), b2=np.asarray(b2, np.float32)[None, :],
        fcW1=np.asarray(fcW1, np.float32),
        fcb1=np.asarray(fcb1, np.float32)[None, :],
        fcW2=np.asarray(fcW2, np.float32),
        fcb2=np.asarray(fcb2, np.float32)[None, :],
    )
    perm = meta["perm"]
    in_maps = []
    for c in range(meta["NC"]):
        d = dict(shared)
        pc = per_core[c]
        # permuted local xs rows (pads -> zero row N)
        pidx = np.where(perm[c] >= 0, perm[c], x.shape[0])
        d["xsp"] = xs_bf[pidx]
        d["idx1"] = pc["idx1"]
        d["idx2"] = pc["idx2"]
        d["dinv_dst"] = pc["dinv_dst"]
        d["scat_rows"] = pc["scat_rows"]
        d["scat_g"] = np.tile(pc["scat_g"], (1, 1))
        in_maps.append(d)

    res = bass_utils.run_bass_kernel_spmd(
        nc, in_maps, core_ids=list(range(meta["NC"])), trace=trace)
    out = res.results[0]["out"]
    kernel.last_exec_time_ns = res.exec_time_ns
    return out


kernel.last_exec_time_ns = None

